# revision 44
# baseline (speedup 1.0000x reference)
"""Multi-head attention (B=2, S=4096, D=768, H=12) on 8 Trainium2 cores.

Sharding: core c -> batch b = c // 4, head-triple g = c % 4 (heads 3g..3g+2).
Each core computes its QKV projections (columns of W for its heads) and
flash-style attention for its 3 heads, fully on-chip; no cross-core comms.
Host-side prep per core: slice batch/head-group, cast x/W to fp16 (the device
kernel computes in fp16 with f32 accumulation; fp16 over bf16 because all
value ranges here are small, giving ~8x lower quantization error at identical
PE throughput; casting host-side also halves the transfer volume).

Per-core device kernel:
  - x^T tiles [128(d), 6(dchunk), 1024(s)] via xbar DMA-transpose straight
    from the fp16 DRAM inputs, quarter by quarter; projections chase each
    quarter so ScalarE attention work starts ~tens of us in.
  - projections on PE produce qT/kT [64, 3, 4096] and v_ext
    [128, 32, 3, 65] (col 64 = ones so the PV matmul accumulates the softmax
    denominator as output row 64). The attention mask enters as a per-k scale
    em = exp(-1e4*(1-mask)) folded into v_ext (exact: softmax with additive
    -1e4 adder == scaling exp(s) by em, including the denominator).
  - attention per (head, 512-wide q chunk): 32 k-chunks in groups of 3;
    QK^T -> PSUM, exp on ScalarE (scale=1/8) -> fp16 probs in SBUF,
    PV accumulate -> PSUM [65, 512]; then PE-transpose to natural layout and
    DVE normalize by the reciprocal of the denominator column (row 64).

Perf structure (HW-verified this series; engine rates from microbenchmarks in
probe.py):
  - QK^T row tiling: odd k-chunks read the qT/kT replicas on partitions
    64-127, so consecutive K=64 matmuls land on different PE row groups and
    run concurrently (589us -> 388us like-for-like). This is the reason for
    the w_dup duplication.
  - split finalize: the pv->SBUF copy is emitted at the NEXT iteration's
    first exp group (ahead of everything else DVE does that iteration) so
    the pv PSUM slot frees early; transposes+normalize one group later so
    they queue behind QK g1 on the PE.
  - fast-start prologue: only the (h0,s0) k+q projections gate the first
    exp group.
  - rejected on HW A/B (kept opt-in): BASS_DVE_EXP=1 offloads 3 k-chunks/
    iter of exp to a DVE deg-6 Horner poly (368us vs 299us best-case);
    BASS_PAIR=1 pairs QK emission across exp-group boundaries (478 vs 450).
  - PSUM budget: scores 2 bufs x 3 banks + pv 1 + tr 1 = 8 (full). GRP=4
    exp groups or 3-deep scores buffering require freeing a bank first.
"""

import os
import sys

if "/opt/trn_rl_repo" not in sys.path:
    sys.path.insert(0, "/opt/trn_rl_repo")

from contextlib import ExitStack

import ml_dtypes
import numpy as np

import concourse.bass as bass
import concourse.tile as tile
from concourse import bacc, mybir
from concourse.bass_utils import run_bass_kernel_spmd
from concourse.masks import make_identity

F32 = mybir.dt.float32
# fp16 instead of bf16: all on-chip value ranges here are tiny (|x|<6,
# |W|<0.12, probs<8), so fp16's 10 mantissa bits cut quantization error ~4x
# at identical PE throughput (1 cycle/row) and xbar 2-byte transpose support
BF16 = mybir.dt.float16
AF = mybir.ActivationFunctionType
ALU = mybir.AluOpType
BF16_NP = np.float16

B, S, D, H, DK = 2, 4096, 768, 12, 64
N_CORES = 8
HPG = 3            # heads per core
GD = HPG * DK      # 192 output columns per core
SQ = 512           # q-chunk width
NSQ = S // SQ      # 8
KCW = 128          # k-chunk width
NKC = S // KCW     # 32
GRP = 3            # k-chunks per exp group (3 PSUM banks, double buffered)
NDC = D // 128     # 6 contraction chunks
QTR = S // 4       # transpose/projection pipeline granularity
SQQ = NSQ // 4     # q chunks per quarter
SCQ = NKC // 4     # s chunks per quarter


def _emit(ctx: ExitStack, tc: tile.TileContext, io: dict):
    nc = tc.nc

    const = ctx.enter_context(tc.tile_pool(name="const", bufs=1))
    # 8 slots: quarters xk0-3/xq0/xv0-2 are all live early; xv3 (9th alloc)
    # then reuses xk0's slot, whose readers finish by ~f6 — reuse of any
    # later slot (e.g. xq0's, read until ~f30) would deadlock the PE queue
    # against iteration 0's PV(g8).
    xt_pool = ctx.enter_context(tc.tile_pool(name="xt", bufs=8))
    proj = ctx.enter_context(tc.tile_pool(name="proj", bufs=1))
    scores_pool = ctx.enter_context(tc.tile_pool(name="scores", bufs=2, space="PSUM"))
    aux_psum = ctx.enter_context(tc.tile_pool(name="auxp", bufs=2, space="PSUM"))
    probs_pool = ctx.enter_context(tc.tile_pool(name="probs", bufs=5))
    outt_pool = ctx.enter_context(tc.tile_pool(name="outt", bufs=2))
    small = ctx.enter_context(tc.tile_pool(name="small", bufs=2))
    oslab_pool = ctx.enter_context(tc.tile_pool(name="oslab", bufs=3))
    # DVE-exp offload scratch: x tile, Horner ping-pong, and probs output
    dx_pool = ctx.enter_context(tc.tile_pool(name="dx", bufs=2))
    dh_pool = ctx.enter_context(tc.tile_pool(name="dh", bufs=3))
    prd_pool = ctx.enter_context(tc.tile_pool(name="prd", bufs=2))

    # ---- constants / small inputs (consolidated to limit 4KB slot padding) ----
    # mask -> per-k scale em = exp(-1e4 * (1 - mask)), [128, 32] (p, kchunk).
    # Emitted FIRST so the ACT exp-table load lands at the head of the queues.
    # All const DMAs go on the SAME queue (SP) that later issues the x
    # DMA-transposes: the shared HWDGE serializes globally and every
    # copy<->transpose transition costs an xbar-mode drain, so mixed-queue
    # issue (copies from ACT, transposes from SP) interleaved them worst-case.
    mask_em = const.tile([128, 65], F32, name="mask_em")
    mask_t = mask_em[:, 0:32]
    em_sb = mask_em[:, 32:64]
    neg1e4 = mask_em[:, 64:65]
    nc.gpsimd.memset(neg1e4, -10000.0)
    nc.sync.dma_start(mask_t, io["mask_pk"][:])
    nc.scalar.activation(em_sb, mask_t, AF.Exp, scale=10000.0, bias=neg1e4)

    # "0": all-fp16 projections. "1": q AND k projections via fp8 DoubleRow
    # (hw-measured end-to-end rel err 1.8e-2 — too close to the 2e-2 gate).
    # "k": only the K projection in fp8 (err ~1.2e-2 l2 / 1.5e-2 absmax,
    # comfortable margin) at half the PE savings.
    FP8MODE = __import__("os").environ.get("BASS_FP8QK", "k")
    FP8QK = FP8MODE in ("1", "k")
    F8 = mybir.dt.float8e4

    def fp8_for(wi):
        return FP8MODE == "1" or (FP8MODE == "k" and wi == 1)

    if FP8QK:
        # q/k weights as fp8 DoubleRow pairs, host-packed in tile layout:
        # (ki, c, i, j, h, m) = 32*W_i[256c + 2ki + j, h*64 + m%64]
        # (x32 lifts W sigma=0.02 out of fp8's subnormal range; the bias-add
        # multiplies the PSUM result by 1/32)
        w8 = const.tile([128, 3, 2, 2, HPG, 128], F8, name="w8")
        nc.sync.dma_start(
            w8[:],
            io["wqk8"].rearrange(
                "p (c i j h m) -> p c i j h m", c=3, i=2, j=2, h=HPG
            ),
        )
    # fp16 weight slabs: v always; q and/or k when their projection is fp16
    fp16_w = [(2, "wv")]
    if not fp8_for(0):
        fp16_w.append((0, "wq"))
    if not fp8_for(1):
        fp16_w.append((1, "wk"))
    w_all = const.tile([128, NDC, 3 * GD], BF16, name="w_all")
    for i, nm in fp16_w:
        nc.sync.dma_start(
            w_all[:, :, i * GD : (i + 1) * GD],
            io[nm].rearrange("(dc p) n -> p dc n", p=128),
        )
    wv_sb = w_all[:, :, 2 * GD : 3 * GD]

    if not (fp8_for(0) and fp8_for(1)):
        # q/k weights with each head's 64 columns duplicated (projection then
        # replicates qT/kT on both partition halves at no extra PE cost)
        w_dup = const.tile([128, NDC, 2, HPG, 128], BF16, name="w_dup")
        for i, _nm in fp16_w:
            if i == 2:
                continue
            for h in range(HPG):
                for rep in range(2):
                    nc.vector.tensor_copy(
                        w_dup[:, :, i, h, rep * DK : (rep + 1) * DK],
                        w_all[:, :, i * GD + h * DK : i * GD + (h + 1) * DK],
                    )

    bqbk = const.tile([128, 2 * HPG], F32, name="bqbk")
    nc.sync.dma_start(bqbk[:], io["bqbk_pk"][:])

    bfpack = const.tile([1, 320], BF16, name="bfpack")
    nc.gpsimd.memset(bfpack[:, 0:128], 1.0)
    nc.sync.dma_start(bfpack[:, 128 : 128 + GD], io["bv_r"][:])
    ones_row = bfpack[:, 0:128]
    bv_sb = bfpack[:, 128 : 128 + GD]

    ident = const.tile([128, 128], F32, name="ident")
    make_identity(nc, ident[:])

    # ---- persistent projection outputs (qT/kT replicated on both halves) ----
    qT = proj.tile([128, HPG, S], BF16, name="qT")
    kT = proj.tile([128, HPG, S], BF16, name="kT")
    vE = proj.tile([128, NKC, HPG, DK + 1], BF16, name="vE")
    nc.gpsimd.memset(vE[:], 1.0)  # ones col 64; data cols overwritten below

    # ---- per-quarter: transpose + project ----
    def load_xt_quarter(nm, qq):
        # host supplies x d-chunk-major [6*4096, 128] so each xbar transpose
        # reads a fully contiguous [1024, 128] block. In FP8QK mode, xq/xk
        # arrive byte-packed (two fp8 d-neighbors per uint16 element): 3
        # chunks of 128 pair-columns, half the DMA bytes.
        packed = (nm == "xk" and FP8QK) or (nm == "xq" and FP8MODE == "1")
        nch = 3 if packed else NDC
        xt = xt_pool.tile([128, nch, QTR], BF16, tag="xt", name=f"xt_{nm}_{qq}")
        for dc in range(nch):
            base = dc * S + qq * QTR
            nc.sync.dma_start(
                out=xt[:, dc, :], in_=io[nm][base : base + QTR, :],
                transpose=True,
            )
        return xt

    def proj_qk_one(xt, qq, wi, bias, dst, h, sqq):
        # scores-pool slot (not aux): aux holds the live pv accumulator, so a
        # second rotating tenant there would serialize every projection
        # against its DVE bias-add read
        sq = qq * SQQ + sqq
        ps = scores_pool.tile([128, SQ], F32, tag="scores", name=f"ps_{wi}_{qq}_{h}_{sqq}")
        if fp8_for(wi):
            # fp8 DoubleRow: 3 contraction chunks of 256 d (pairs d=256c+2ki+j
            # matching the byte-packed transpose and the host w8 layout);
            # each chunk streams N=512 at 0.5 cycles/row
            x8 = xt[:].bitcast(F8).rearrange("p c (s j) -> p c j s", j=2)
            for c in range(3):
                nc.tensor.matmul(
                    ps[:],
                    lhsT=w8[:, c, wi, :, h, :],
                    rhs=x8[:, c, :, sqq * SQ : (sqq + 1) * SQ],
                    start=(c == 0),
                    stop=(c == 2),
                    perf_mode=mybir.MatmulPerfMode.DoubleRow,
                )
            # undo the x32 weight scale, then add bias
            nc.vector.tensor_scalar(
                dst[:, h, sq * SQ : (sq + 1) * SQ], ps[:],
                0.03125, bias[:, h : h + 1], ALU.mult, ALU.add,
            )
        else:
            for dc in range(NDC):
                nc.tensor.matmul(
                    ps[:],
                    lhsT=w_dup[:, dc, wi, h, :],
                    rhs=xt[:, dc, sqq * SQ : (sqq + 1) * SQ],
                    start=(dc == 0),
                    stop=(dc == NDC - 1),
                )
            nc.vector.tensor_scalar(
                dst[:, h, sq * SQ : (sq + 1) * SQ], ps[:],
                bias[:, h : h + 1], None, ALU.add,
            )

    def proj_qk(xt, qq, wi, bias, dst, skip=None):
        for h in range(HPG):
            for sqq in range(SQQ):
                if skip is not None and (h, sqq) in skip:
                    continue
                proj_qk_one(xt, qq, wi, bias, dst, h, sqq)

    def proj_v(xt, qq, scqs=None):
        for scq in (range(SCQ) if scqs is None else scqs):
            sc = qq * SCQ + scq
            ps = scores_pool.tile([128, GD], F32, tag="scores", name=f"psv_{qq}_{scq}")
            for dc in range(NDC):
                nc.tensor.matmul(
                    ps[:],
                    lhsT=xt[:, dc, scq * 128 : (scq + 1) * 128],
                    rhs=wv_sb[:, dc, :],
                    start=(dc == 0),
                    stop=False,
                )
            nc.tensor.matmul(
                ps[:], lhsT=ones_row[:, 0:128], rhs=bv_sb[:], start=False, stop=True
            )
            for h in range(HPG):
                nc.vector.tensor_copy(
                    vE[:, sc, h, 0:DK], ps[:, h * DK : (h + 1) * DK]
                )
            # fold mask scale into v and the denominator ones column
            nc.vector.tensor_scalar(
                vE[:, sc, :, :], vE[:, sc, :, :], em_sb[:, sc : sc + 1], None,
                ALU.mult,
            )

    def proj_q_group(xt, qq, h, sqq):
        proj_qk_one(xt, qq, 0, bqbk[:, 0:HPG], qT, h, sqq)

    # ---- attention ----
    groups = []
    g0 = 0
    while g0 < NKC:
        groups.append((g0, min(GRP, NKC - g0)))
        g0 += GRP

    # ScalarE exp is the kernel's critical engine in steady state. Offload
    # the LAST `DVE_NGROUPS` exp groups of each iteration (latest PV
    # deadlines) to a DVE polynomial: exp(s/8) = h(y)^4 with y = s/32 and
    # h a degree-2 least-squares fit of exp on y in [-0.5, 0.5]. Squaring
    # twice keeps probs nonnegative by construction. 6 DVE ops per group
    # (1 PSUM-read + 5 fp16-SBUF ops) vs 1 ACT op; worth it because DVE is
    # ~80% idle while ACT is the wall. Adds ~3.7e-3 relative error on the
    # offloaded 5/32 of keys (budget 2e-2).
    # A/B on HW (same-process, interleaved): offload=2 groups measured ~26us
    # SLOWER (579 vs 553) despite ACT being the busiest engine — the DVE
    # chain latency sits on the iteration-transition critical path. Off by
    # default.
    DVE_NGROUPS = int(__import__("os").environ.get("BASS_DVE_NGROUPS", "0"))
    C0, C1, C2 = 1.00148143, 1.02379966, 0.48757841

    def emit_dve_exp(sc, pr, w, it, gi):
        yt = dx_pool.tile([128, w], BF16, tag="dx", name=f"y_{it}_{gi}")
        nc.vector.tensor_scalar(yt[:], sc[:], 0.03125, None, ALU.mult)
        t1 = dh_pool.tile([128, w], BF16, tag="dh", name=f"t1_{it}_{gi}")
        nc.vector.tensor_scalar(t1[:], yt[:], C2, C1, ALU.mult, ALU.add)
        t2 = dh_pool.tile([128, w], BF16, tag="dh", name=f"t2_{it}_{gi}")
        nc.vector.tensor_tensor(t2[:], t1[:], yt[:], ALU.mult)
        t3 = dh_pool.tile([128, w], BF16, tag="dh", name=f"t3_{it}_{gi}")
        nc.vector.tensor_scalar(t3[:], t2[:], C0, None, ALU.add)
        t4 = dh_pool.tile([128, w], BF16, tag="dh", name=f"t4_{it}_{gi}")
        nc.vector.tensor_tensor(t4[:], t3[:], t3[:], ALU.mult)
        nc.vector.tensor_tensor(pr[:], t4[:], t4[:], ALU.mult)

    pending = None  # finalize closure for the previous (h, sq)

    # finalize split in two: part 1 (the pv->SBUF copy, which frees the pv
    # PSUM slot) fires at gi==0 so it lands in the DVE queue BEFORE the
    # ~10us exp-poly chain; part 2 (PE transposes + normalize + store) at
    # gi==1 so the transposes sit behind QK g1 in the PE queue and never
    # stall on the copy.
    def make_fin_copy(pv, h, sq):
        ot = outt_pool.tile([DK + 1, SQ], F32, tag="outt", name=f"ot_{h}_{sq}")

        def fin1():
            nc.vector.tensor_copy(ot[:], pv[:])
        return ot, fin1

    def make_finalize(ot, h, sq):
        def fin():
            tr = aux_psum.tile([128, 4 * (DK + 1)], F32, tag="aux", name=f"tr_{h}_{sq}")
            for t in range(4):
                nc.tensor.transpose(
                    tr[:, t * (DK + 1) : (t + 1) * (DK + 1)],
                    ot[:, t * 128 : (t + 1) * 128],
                    ident[0 : DK + 1, 0 : DK + 1],
                )
            rc = small.tile([128, 4], F32, tag="recip", name=f"rc_{h}_{sq}")
            osl = oslab_pool.tile([128, 4, DK], F32, tag="oslab", name=f"os_{h}_{sq}")
            for t in range(4):
                nc.vector.reciprocal(
                    rc[:, t : t + 1], tr[:, t * (DK + 1) + DK : t * (DK + 1) + DK + 1]
                )
                nc.vector.tensor_scalar(
                    osl[:, t, :],
                    tr[:, t * (DK + 1) : t * (DK + 1) + DK],
                    rc[:, t : t + 1],
                    None,
                    ALU.mult,
                )
            nc.gpsimd.dma_start(
                out=io["out"].rearrange(
                    "(sq t p) n -> sq p t n", sq=NSQ, t=4, p=128
                )[sq, :, :, h * DK : (h + 1) * DK],
                in_=osl[:],
            )
        return fin

    # Boundary tasks: kept as an (empty by default) hook used by the carry
    # emission point inside attention_gen.
    boundary_tasks = {}
    xt_q_tiles = {}

    def attention_gen():
        nonlocal_pending = [None]

        def emit_pv(pv, h, grp, pvst):
            # start/stop are positional (pvst counts PV matmuls emitted for
            # this accumulator): chunk emission order is permuted when DVE
            # exp groups are deferred, so kc == 0 is not necessarily first.
            p0, plen, ppr = grp
            for j in range(plen):
                kc = p0 + j
                nc.tensor.matmul(
                    pv[:],
                    lhsT=vE[:, kc, h, :],
                    rhs=ppr[:, j * SQ : (j + 1) * SQ],
                    start=(pvst[0] == 0),
                    stop=(pvst[0] == NKC - 1),
                )
                pvst[0] += 1

        carry = None  # (pv, h, [groups]) tail-PV work carried across iterations
        it = 0
        ITER_HMINOR = __import__("os").environ.get("BASS_HMINOR", "1") == "1"
        # h-minor / sq-major order: qT quarter q is first needed at iteration
        # 3*2q, so the xq DMAs and qT projections can trail far behind the
        # kT/vE pipeline instead of gating it.
        if ITER_HMINOR:
            iters = [(h, sq) for sq in range(NSQ) for h in range(HPG)]
        else:
            iters = [(h, sq) for h in range(HPG) for sq in range(NSQ)]
        for h, sq in iters:
            # From iteration 2 on, the DVE-offloaded groups (the LAST k-chunk
            # groups) are FIRED FIRST: their scores slots free early (from the
            # previous iteration), and their ~5us DVE poly chains overlap this
            # iteration's ACT burst instead of delaying the carry PVs.
            offload = DVE_NGROUPS if it >= 2 else 0
            if offload:
                act_groups = groups[-offload:] + groups[:-offload]
            else:
                act_groups = groups
            dve_set = set(range(offload))
            pv = aux_psum.tile([DK + 1, SQ], F32, tag="aux", name=f"pv_{h}_{sq}")
            pvst = [0]
            ready = []  # (kc0, glen, probs) groups awaiting PV emission
            defer = []  # (eligible_fire_idx, grp) DVE groups awaiting poly
            # QK matmuls are emitted in strict (even, odd) kc pairs ACROSS
            # group boundaries so every matmul lands adjacent to its
            # opposite-row-group partner in the PE queue and the two K=64
            # halves run concurrently (PV blocks between groups would
            # otherwise orphan each group's 3rd chunk)
            chunk_list = []
            for gi, (kc0, glen) in enumerate(act_groups):
                for j in range(glen):
                    chunk_list.append((kc0 + j, gi, j))
            sc_tiles = {}
            filled = [0] * len(act_groups)
            fired = 0
            ci = 0
            # no pairing in iteration 0: its one-chunk lookahead would hold a
            # live scores tile across the prologue's advance() points, where
            # proj_q_group borrows slots from the same pool
            # A/B on HW: pairing measured neutral-to-worse (478us vs 450us
            # best-valid samples) — likely the 1-chunk lookahead couples the
            # PE to the previous exp via the scores double-buffer. Opt-in.
            do_pair = __import__("os").environ.get("BASS_PAIR", "0") == "1"
            pair_n = 2 if (it > 0 and do_pair) else 1
            while ci < len(chunk_list):
                for _ in range(pair_n):
                    if ci >= len(chunk_list):
                        break
                    kc, gi, j = chunk_list[ci]
                    ci += 1
                    if gi not in sc_tiles:
                        sc_tiles[gi] = scores_pool.tile(
                            [128, act_groups[gi][1] * SQ], F32, tag="scores",
                            name=f"sc_{h}_{sq}_{gi}",
                        )
                    ho = 64 * (kc % 2)
                    nc.tensor.matmul(
                        sc_tiles[gi][:, j * SQ : (j + 1) * SQ],
                        lhsT=kT[ho : ho + DK, h, kc * KCW : (kc + 1) * KCW],
                        rhs=qT[ho : ho + DK, h, sq * SQ : (sq + 1) * SQ],
                        start=True,
                        stop=True,
                    )
                    filled[gi] += 1
                while fired < len(act_groups) and (
                    filled[fired] == act_groups[fired][1]
                ):
                    gi = fired
                    kc0, glen = act_groups[gi]
                    pr = probs_pool.tile(
                        [128, glen * SQ], BF16, tag="probs",
                        name=f"pr_{h}_{sq}_{gi}",
                    )
                    if gi in dve_set:
                        emit_dve_exp(
                            sc_tiles.pop(gi), pr, glen * SQ, it, gi
                        )
                        defer.append((gi + 4, (kc0, glen, pr)))
                    else:
                        nc.scalar.activation(
                            pr[:], sc_tiles.pop(gi)[:], AF.Exp, scale=0.125
                        )
                        ready.append((kc0, glen, pr))
                    while defer and defer[0][0] <= gi:
                        ready.append(defer.pop(0)[1])
                    if gi == 0:
                        if carry is not None:
                            cpv, ch, cgrps, cpvst = carry
                            for grp in cgrps:
                                emit_pv(cpv, ch, grp, cpvst)
                            carry = None
                            for fn in boundary_tasks.get(it - 1, ()):
                                fn()
                        # free the previous pv PSUM slot (DVE copy) BEFORE
                        # the iteration's poly chains enter the DVE queue
                        if nonlocal_pending[0] is not None:
                            nonlocal_pending[0][0]()
                    if gi == 1 and nonlocal_pending[0] is not None:
                        nonlocal_pending[0][1]()
                        nonlocal_pending[0] = None
                    # iteration 0 holds one extra group in flight: its vE
                    # quarters are still streaming in, so the deeper lag keeps
                    # PV emission behind the xv DMA wavefront
                    if len(ready) >= (3 if it == 0 else 2):
                        emit_pv(pv, h, ready.pop(0), pvst)
                    fired += 1
                    yield (h, sq, gi)
            carry_grps = list(ready) + [g for _, g in defer]
            carry = (pv, h, carry_grps, pvst)
            ot, fin1 = make_fin_copy(pv, h, sq)
            nonlocal_pending[0] = (fin1, make_finalize(ot, h, sq))
            it += 1

        cpv, ch, cgrps, cpvst = carry
        for grp in cgrps:
            emit_pv(cpv, ch, grp, cpvst)
        nonlocal_pending[0][0]()
        nonlocal_pending[0][1]()

    # All x DMA-transposes are issued upfront in availability order so the
    # DMA engines stream continuously from t=0 (the quarter-chasing scheme
    # left them ~50% idle over a 100us window). kT quarters land first (they
    # gate iteration 0's exp groups), vE quarters interleave (they gate the
    # trailing PV), and xq quarters come last (h-minor iteration order defers
    # the first sq>=2 iteration far enough that qT projection can trail).
    gen = attention_gen()

    def advance(n):
        for _ in range(n):
            if next(gen, None) is None:
                break

    bk = bqbk[:, HPG : 2 * HPG]
    xt_k0 = load_xt_quarter("xk", 0)
    xt_q_tiles[0] = load_xt_quarter("xq", 0)
    xt_v0 = load_xt_quarter("xv", 0)
    xt_k1 = load_xt_quarter("xk", 1)
    xt_v1 = load_xt_quarter("xv", 1)
    xt_k2 = load_xt_quarter("xk", 2)
    xt_v2 = load_xt_quarter("xv", 2)
    xt_k3 = load_xt_quarter("xk", 3)
    # (xv3 + xq1..3 are issued below once xt slots have been consumed)

    # Fine-grained prologue: projections are emitted in DMA-availability
    # order, spread across iteration 0's fires so the PE never holds a long
    # burst ahead of runnable attention work, and per-(head, sqq) so only
    # head 0's kT gates iteration 0 (h1/h2 trail for iterations 1-2; all xt
    # readers still finish within iteration 0's span, freeing slots for
    # xv3/xq1-3).
    # PV(g) of iteration 0 is emitted at fire g+3 (pop threshold 3), so vE
    # chunks 3g..3g+2 must be emitted before fire g+3.
    proj_qk_one(xt_k0, 0, 1, bk, kT, 0, 0)
    proj_q_group(xt_q_tiles[0], 0, 0, 0)
    advance(1)   # f1: g0 (chunks 0-2)
    proj_qk_one(xt_k0, 0, 1, bk, kT, 0, 1)
    proj_qk_one(xt_k0, 0, 1, bk, kT, 1, 0)
    advance(1)   # f2: g1
    proj_v(xt_v0, 0, (0, 1, 2))
    proj_qk_one(xt_k1, 1, 1, bk, kT, 0, 0)    # g2's chunk 8
    advance(1)   # f3: g2 + PV(g0)
    proj_v(xt_v0, 0, (3, 4, 5))
    proj_qk_one(xt_k0, 0, 1, bk, kT, 1, 1)
    advance(1)   # f4: g3 + PV(g1)
    proj_v(xt_v0, 0, (6, 7))
    proj_v(xt_v1, 1, (0,))
    proj_qk_one(xt_k1, 1, 1, bk, kT, 0, 1)    # g4's chunks 12-14
    proj_qk_one(xt_k0, 0, 1, bk, kT, 2, 0)
    advance(1)   # f5: g4 + PV(g2)
    proj_v(xt_v1, 1, (1, 2, 3))
    proj_qk_one(xt_k2, 2, 1, bk, kT, 0, 0)    # g5's chunks 16-17
    proj_qk_one(xt_k0, 0, 1, bk, kT, 2, 1)
    advance(1)   # f6: g5 + PV(g3)
    proj_v(xt_v1, 1, (4, 5, 6))
    proj_qk_one(xt_k2, 2, 1, bk, kT, 0, 1)    # g6's chunk 20
    proj_qk_one(xt_k1, 1, 1, bk, kT, 1, 0)
    advance(1)   # f7: g6 + PV(g4)
    xt_v3 = load_xt_quarter("xv", 3)
    proj_v(xt_v1, 1, (7,))
    proj_v(xt_v2, 2, (0, 1))
    proj_qk_one(xt_k1, 1, 1, bk, kT, 1, 1)
    advance(1)   # f8: g7 + PV(g5)
    proj_v(xt_v2, 2, (2, 3, 4))
    proj_qk_one(xt_k3, 3, 1, bk, kT, 0, 0)    # g8's chunk 24
    proj_qk_one(xt_k1, 1, 1, bk, kT, 2, 0)
    advance(1)   # f9: g8 + PV(g6)
    proj_v(xt_v2, 2, (5, 6, 7))
    proj_qk_one(xt_k3, 3, 1, bk, kT, 0, 1)    # g9's chunk 28
    proj_qk_one(xt_k1, 1, 1, bk, kT, 2, 1)
    xt_q_tiles[1] = load_xt_quarter("xq", 1)
    advance(1)   # f10: g9 + PV(g7)
    proj_v(xt_v3, 3, (0, 1, 2))
    proj_qk_one(xt_k2, 2, 1, bk, kT, 1, 0)
    proj_qk_one(xt_k2, 2, 1, bk, kT, 1, 1)
    xt_q_tiles[2] = load_xt_quarter("xq", 2)
    advance(1)   # f11: g10 + PV(g8) — iteration 0 groups complete
    proj_v(xt_v3, 3, (3, 4, 5, 6, 7))
    proj_q_group(xt_q_tiles[0], 0, 1, 0)      # iter1 = (h1, s0) at f12
    proj_qk_one(xt_k2, 2, 1, bk, kT, 2, 0)
    proj_qk_one(xt_k2, 2, 1, bk, kT, 2, 1)
    xt_q_tiles[3] = load_xt_quarter("xq", 3)
    advance(2)   # f12 (iter1 g0: carry PVs g9,g10 + fin), f13
    proj_qk_one(xt_k3, 3, 1, bk, kT, 1, 0)    # iter1 g8 reads at f20
    proj_qk_one(xt_k3, 3, 1, bk, kT, 1, 1)
    advance(2)   # f14, f15
    proj_qk_one(xt_k3, 3, 1, bk, kT, 2, 0)    # iter2 g8 reads at f31
    proj_qk_one(xt_k3, 3, 1, bk, kT, 2, 1)
    proj_q_group(xt_q_tiles[0], 0, 2, 0)      # iter2 = (h2, s0) at f23
    advance(7)   # f16-f22
    proj_q_group(xt_q_tiles[0], 0, 0, 1)      # iter3 = (h0, s1) at f34
    advance(2)   # f23, f24
    proj_q_group(xt_q_tiles[0], 0, 1, 1)      # iter4 at f45
    advance(5)   # f25-f29
    proj_q_group(xt_q_tiles[0], 0, 2, 1)      # iter5 at f56
    advance(5)   # f30-f34
    # qT quarters 1-3: one projection per 5 fires from ~f35 (deadline for
    # quarter q head h is fire 11*(6q+h), loose for every entry)
    for qq in (1, 2, 3):
        for h in range(HPG):
            for sqq in range(SQQ):
                proj_q_group(xt_q_tiles[qq], qq, h, sqq)
                advance(5)
    for _ in gen:
        pass


def _build():
    nc = bacc.Bacc("TRN2", target_bir_lowering=False, debug=False)
    mode = os.environ.get("BASS_FP8QK", "k")
    io = {}
    tensors = [
        ("xv", [NDC * S, 128], BF16), ("wv", [D, GD], BF16),
        ("bqbk_pk", [128, 2 * HPG], F32),
        ("bv_r", [1, GD], BF16), ("mask_pk", [128, NKC], F32),
    ]
    if mode in ("1", "k"):
        tensors.append(("wqk8", [128, 3 * 2 * 2 * HPG * 128], mybir.dt.float8e4))
        tensors.append(("xk", [3 * S, 128], BF16))
    else:
        tensors += [("xk", [NDC * S, 128], BF16), ("wk", [D, GD], BF16)]
    if mode == "1":
        tensors.append(("xq", [3 * S, 128], BF16))
    else:
        tensors += [("xq", [NDC * S, 128], BF16), ("wq", [D, GD], BF16)]
    for nm, shape, dt in tensors:
        io[nm] = nc.dram_tensor(nm, shape, dt, kind="ExternalInput").ap()
    io["out"] = nc.dram_tensor("out", [S, GD], F32, kind="ExternalOutput").ap()

    dup = int(os.environ.get("BASS_DUP", "1"))
    with tile.TileContext(nc) as tc:
        for _ in range(dup):
            with ExitStack() as ctx:
                _emit(ctx, tc, io)
    nc.compile()
    return nc


_NC = None


def _get_nc():
    global _NC
    if _NC is None:
        _NC = _build()
    return _NC


F8_NP = None


def _f8np():
    global F8_NP
    if F8_NP is None:
        from concourse import mybir as _mb
        F8_NP = _mb.dt.np(_mb.dt.float8e4)
    return F8_NP


def _pack_x8(a):
    # [S, D] f32 -> fp8 -> byte-pair uint16 carrier, d-pair-chunk-major
    # [3*S, 128] viewed as fp16 for the 2-byte xbar transpose
    a8 = np.ascontiguousarray(np.asarray(a, np.float32)).astype(_f8np())
    u = a8.view(np.uint8).reshape(S, 384, 2).view(np.uint16).reshape(S, 3, 128)
    return np.ascontiguousarray(
        u.transpose(1, 0, 2).reshape(3 * S, 128)
    ).view(np.float16)


def _pack_wqk8(Wq_c, Wk_c):
    # (ki, c, i, j, h, m) = 32 * W_i[256c + 2ki + j, h*64 + m%64]
    out = np.empty((128, 3, 2, 2, HPG, 128), np.float32)
    for i, W in ((0, Wq_c), (1, Wk_c)):
        Wr = (np.asarray(W, np.float32) * 32.0).reshape(3, 128, 2, HPG, DK)
        Wm = np.concatenate([Wr, Wr], axis=-1)        # [c, ki, j, h, 128]
        out[:, :, i] = Wm.transpose(1, 0, 2, 3, 4)    # [ki, c, j, h, 128]
    return np.ascontiguousarray(out.reshape(128, -1)).astype(_f8np())


def make_in_maps(query, key, value, mask, Wq, bq, Wk, bk, Wv, bv):
    mode = os.environ.get("BASS_FP8QK", "k")
    bf = lambda a: np.ascontiguousarray(a).astype(BF16_NP)
    bf3 = lambda a: np.ascontiguousarray(
        np.asarray(a).reshape(S, NDC, 128).transpose(1, 0, 2).reshape(NDC * S, 128)
    ).astype(BF16_NP)
    f32 = lambda a: np.ascontiguousarray(np.asarray(a, np.float32))
    in_maps = []
    for c in range(N_CORES):
        b, g = divmod(c, 4)
        cols = slice(g * GD, (g + 1) * GD)
        m = {
            "xv": bf3(value[b]),
            "wv": bf(Wv[:, cols]),
            "bqbk_pk": f32(np.tile(np.concatenate(
                [np.asarray(bq)[cols].reshape(HPG, DK).T,
                 np.asarray(bk)[cols].reshape(HPG, DK).T], axis=1), (2, 1))),
            "bv_r": bf(np.asarray(bv)[cols].reshape(1, GD)),
            "mask_pk": f32(np.asarray(mask)[b].reshape(NKC, 128).T),
        }
        if mode in ("1", "k"):
            m["xk"] = _pack_x8(key[b])
            m["wqk8"] = _pack_wqk8(
                np.asarray(Wq)[:, cols], np.asarray(Wk)[:, cols]
            )
        else:
            m["xk"] = bf3(key[b])
            m["wk"] = bf(Wk[:, cols])
        if mode == "1":
            m["xq"] = _pack_x8(query[b])
        else:
            m["xq"] = bf3(query[b])
            m["wq"] = bf(Wq[:, cols])
        in_maps.append(m)
    return in_maps


def kernel(query, key, value, mask, Wq, bq, Wk, bk, Wv, bv):
    query = np.asarray(query, np.float32)
    key = np.asarray(key, np.float32)
    value = np.asarray(value, np.float32)
    nc = _get_nc()
    in_maps = make_in_maps(query, key, value, mask, Wq, bq, Wk, bk, Wv, bv)
    res = run_bass_kernel_spmd(nc, in_maps, core_ids=list(range(N_CORES)))
    out = np.empty((B, S, D), np.float32)
    for c in range(N_CORES):
        b, g = divmod(c, 4)
        out[b, :, g * GD : (g + 1) * GD] = res.results[c]["out"]
    return out



# revision 45
# speedup vs baseline: 1.1011x; 1.1011x over previous
"""Multi-head attention (B=2, S=4096, D=768, H=12) on 8 Trainium2 cores.

Sharding: core c -> batch b = c // 4, head-triple g = c % 4 (heads 3g..3g+2).
Each core computes its QKV projections (columns of W for its heads) and
flash-style attention for its 3 heads, fully on-chip; no cross-core comms.
Host-side prep per core: slice batch/head-group, cast x/W to fp16 (f32
accumulation on device; fp16 over bf16 because all value ranges here are
tiny, ~8x lower quantization error at identical PE throughput). The K
projection inputs additionally go to fp8 (see below).

Per-core device kernel:
  - x^T tiles via xbar DMA-transpose straight from DRAM; ALL transposes are
    issued upfront in availability order (copies first on the same SP queue
    — mixed-queue issue interleaves copy/transpose at the shared HWDGE and
    every xbar-mode flip costs a drain), so the DMA engines stream
    continuously: kT quarters first, vE interleaved, xq last.
  - projections on PE produce qT/kT [64, 3, 4096] fp16 (duplicated on both
    partition halves) and v_ext [128, 32, 3, 65] (col 64 = ones so the PV
    matmul accumulates the softmax denominator as output row 64). The mask
    enters as a per-k scale em = exp(-1e4*(1-mask)) folded into v_ext
    (exact, including the denominator).
  - K projection (BASS_FP8QK="k", default) runs in fp8e4m3 DoubleRow: the
    host packs adjacent-d pairs of fp8 x into uint16 so the 2-byte xbar
    transpose yields the [128, 2, s] pair layout, and W*32 (lifted out of
    fp8's subnormal range; un-scaled in the bias-add) packed to match.
    Halves xk DMA bytes and K-proj PE streaming. Measured end-to-end err
    1.27e-2 l2 / 1.61e-2 absmax vs the 2e-2 gate. "1" extends it to Q
    (another -11us, but err 1.80e-2 l2 / 2.4e-2 absmax — too close).
  - attention iterations (h, sq) in h-minor order (sq-major), so qT quarter
    q is first needed at iteration 6q and the xq DMAs + qT projections trail
    far behind the kT/vE pipeline. Per iteration: 32 k-chunks in groups of
    3; QK^T -> PSUM, exp on ScalarE (scale=1/8) -> fp16 probs in SBUF, PV
    accumulate -> PSUM [65, 512] (positional start/stop flags); then
    PE-transpose and DVE normalize by the reciprocal of the denominator row.
  - prologue: projections are emitted fine-grained (per head/sqq; vE in
    chunk triplets) interleaved with iteration 0's exp groups, tracking DMA
    arrival; only (h0,s0)'s k+q projections gate the first exp. Iteration
    0 uses PV pop-threshold 3 (vE still streaming); trailing qT projections
    spread one per 5 exp groups through iterations ~2-10.

Perf notes (HW-verified this series):
  - QK^T row tiling: odd k-chunks read the qT/kT replicas on partitions
    64-127 so consecutive K=64 matmuls run on different PE row groups
    (historically 589us -> 388us); reason for the column duplication.
  - split finalize: the pv->SBUF copy is emitted at the NEXT iteration's
    first exp group so the pv PSUM slot frees early; transposes+normalize
    one group later so they queue behind QK g1 on the PE.
  - projection PSUM tiles borrow scores-pool slots (NOT aux): aux holds the
    live pv accumulator, and a second rotating aux tenant serializes every
    projection against its DVE bias-add.
  - rejected on same-process HW A/B: DVE exp offload (BASS_DVE_NGROUPS=2,
    deg-2+double-squaring poly on the last 2 groups/iter) measured 579 vs
    553us — the DVE chain latency lands on the iteration transition.
    BASS_PAIR=1 (QK emission pairing across group boundaries) also negative.
  - PSUM budget: scores 2 bufs x 3 banks + pv 1 + tr 1 = 8 (full).
  - measurement: cross-process HW timing drifts ~+-8%; only same-process
    interleaved A/B slopes (ab.py / ab2.py) are trustworthy.
"""

import os
import sys

if "/opt/trn_rl_repo" not in sys.path:
    sys.path.insert(0, "/opt/trn_rl_repo")

from contextlib import ExitStack

import ml_dtypes
import numpy as np

import concourse.bass as bass
import concourse.tile as tile
from concourse import bacc, mybir
from concourse.bass_utils import run_bass_kernel_spmd
from concourse.masks import make_identity

F32 = mybir.dt.float32
# fp16 instead of bf16: all on-chip value ranges here are tiny (|x|<6,
# |W|<0.12, probs<8), so fp16's 10 mantissa bits cut quantization error ~4x
# at identical PE throughput (1 cycle/row) and xbar 2-byte transpose support
BF16 = mybir.dt.float16
AF = mybir.ActivationFunctionType
ALU = mybir.AluOpType
BF16_NP = np.float16

B, S, D, H, DK = 2, 4096, 768, 12, 64
N_CORES = 8
HPG = 3            # heads per core
GD = HPG * DK      # 192 output columns per core
SQ = 512           # q-chunk width
NSQ = S // SQ      # 8
KCW = 128          # k-chunk width
NKC = S // KCW     # 32
GRP = 3            # k-chunks per exp group (3 PSUM banks, double buffered)
NDC = D // 128     # 6 contraction chunks
QTR = S // 4       # transpose/projection pipeline granularity
SQQ = NSQ // 4     # q chunks per quarter
SCQ = NKC // 4     # s chunks per quarter


def _emit(ctx: ExitStack, tc: tile.TileContext, io: dict):
    nc = tc.nc

    const = ctx.enter_context(tc.tile_pool(name="const", bufs=1))
    # 8 slots: quarters xk0-3/xq0/xv0-2 are all live early; xv3 (9th alloc)
    # then reuses xk0's slot, whose readers finish by ~f6 — reuse of any
    # later slot (e.g. xq0's, read until ~f30) would deadlock the PE queue
    # against iteration 0's PV(g8).
    xt_pool = ctx.enter_context(tc.tile_pool(name="xt", bufs=8))
    proj = ctx.enter_context(tc.tile_pool(name="proj", bufs=1))
    scores_pool = ctx.enter_context(tc.tile_pool(name="scores", bufs=2, space="PSUM"))
    aux_psum = ctx.enter_context(tc.tile_pool(name="auxp", bufs=2, space="PSUM"))
    probs_pool = ctx.enter_context(tc.tile_pool(name="probs", bufs=5))
    outt_pool = ctx.enter_context(tc.tile_pool(name="outt", bufs=2))
    small = ctx.enter_context(tc.tile_pool(name="small", bufs=2))
    oslab_pool = ctx.enter_context(tc.tile_pool(name="oslab", bufs=3))
    # DVE-exp offload scratch: x tile, Horner ping-pong, and probs output
    dx_pool = ctx.enter_context(tc.tile_pool(name="dx", bufs=2))
    dh_pool = ctx.enter_context(tc.tile_pool(name="dh", bufs=3))
    prd_pool = ctx.enter_context(tc.tile_pool(name="prd", bufs=2))

    # ---- constants / small inputs (consolidated to limit 4KB slot padding) ----
    # mask -> per-k scale em = exp(-1e4 * (1 - mask)), [128, 32] (p, kchunk).
    # Emitted FIRST so the ACT exp-table load lands at the head of the queues.
    # All const DMAs go on the SAME queue (SP) that later issues the x
    # DMA-transposes: the shared HWDGE serializes globally and every
    # copy<->transpose transition costs an xbar-mode drain, so mixed-queue
    # issue (copies from ACT, transposes from SP) interleaved them worst-case.
    mask_em = const.tile([128, 65], F32, name="mask_em")
    mask_t = mask_em[:, 0:32]
    em_sb = mask_em[:, 32:64]
    neg1e4 = mask_em[:, 64:65]
    nc.gpsimd.memset(neg1e4, -10000.0)
    nc.sync.dma_start(mask_t, io["mask_pk"][:])
    nc.scalar.activation(em_sb, mask_t, AF.Exp, scale=10000.0, bias=neg1e4)

    # "0": all-fp16 projections. "1": q AND k projections via fp8 DoubleRow
    # (hw-measured end-to-end rel err 1.8e-2 — too close to the 2e-2 gate).
    # "k": only the K projection in fp8 (err ~1.2e-2 l2 / 1.5e-2 absmax,
    # comfortable margin) at half the PE savings.
    FP8MODE = __import__("os").environ.get("BASS_FP8QK", "k")
    FP8QK = FP8MODE in ("1", "k")
    F8 = mybir.dt.float8e4

    def fp8_for(wi):
        return FP8MODE == "1" or (FP8MODE == "k" and wi == 1)

    if FP8QK:
        # q/k weights as fp8 DoubleRow pairs, host-packed in tile layout:
        # (ki, c, i, j, h, m) = 32*W_i[256c + 2ki + j, h*64 + m%64]
        # (x32 lifts W sigma=0.02 out of fp8's subnormal range; the bias-add
        # multiplies the PSUM result by 1/32)
        w8 = const.tile([128, 3, 2, 2, HPG, 128], F8, name="w8")
        nc.sync.dma_start(
            w8[:],
            io["wqk8"].rearrange(
                "p (c i j h m) -> p c i j h m", c=3, i=2, j=2, h=HPG
            ),
        )
    # fp16 weight slabs: v always; q and/or k when their projection is fp16
    fp16_w = [(2, "wv")]
    if not fp8_for(0):
        fp16_w.append((0, "wq"))
    if not fp8_for(1):
        fp16_w.append((1, "wk"))
    w_all = const.tile([128, NDC, 3 * GD], BF16, name="w_all")
    for i, nm in fp16_w:
        nc.sync.dma_start(
            w_all[:, :, i * GD : (i + 1) * GD],
            io[nm].rearrange("(dc p) n -> p dc n", p=128),
        )
    wv_sb = w_all[:, :, 2 * GD : 3 * GD]

    if not (fp8_for(0) and fp8_for(1)):
        # q/k weights with each head's 64 columns duplicated (projection then
        # replicates qT/kT on both partition halves at no extra PE cost)
        w_dup = const.tile([128, NDC, 2, HPG, 128], BF16, name="w_dup")
        for i, _nm in fp16_w:
            if i == 2:
                continue
            for h in range(HPG):
                for rep in range(2):
                    nc.vector.tensor_copy(
                        w_dup[:, :, i, h, rep * DK : (rep + 1) * DK],
                        w_all[:, :, i * GD + h * DK : i * GD + (h + 1) * DK],
                    )

    bqbk = const.tile([128, 2 * HPG], F32, name="bqbk")
    nc.sync.dma_start(bqbk[:], io["bqbk_pk"][:])

    bfpack = const.tile([1, 320], BF16, name="bfpack")
    nc.gpsimd.memset(bfpack[:, 0:128], 1.0)
    nc.sync.dma_start(bfpack[:, 128 : 128 + GD], io["bv_r"][:])
    ones_row = bfpack[:, 0:128]
    bv_sb = bfpack[:, 128 : 128 + GD]

    ident = const.tile([128, 128], F32, name="ident")
    make_identity(nc, ident[:])

    # ---- persistent projection outputs (qT/kT replicated on both halves) ----
    qT = proj.tile([128, HPG, S], BF16, name="qT")
    kT = proj.tile([128, HPG, S], BF16, name="kT")
    vE = proj.tile([128, NKC, HPG, DK + 1], BF16, name="vE")
    nc.gpsimd.memset(vE[:], 1.0)  # ones col 64; data cols overwritten below

    # ---- per-quarter: transpose + project ----
    def load_xt_quarter(nm, qq):
        # host supplies x d-chunk-major [6*4096, 128] so each xbar transpose
        # reads a fully contiguous [1024, 128] block. In FP8QK mode, xq/xk
        # arrive byte-packed (two fp8 d-neighbors per uint16 element): 3
        # chunks of 128 pair-columns, half the DMA bytes.
        packed = (nm == "xk" and FP8QK) or (nm == "xq" and FP8MODE == "1")
        nch = 3 if packed else NDC
        xt = xt_pool.tile([128, nch, QTR], BF16, tag="xt", name=f"xt_{nm}_{qq}")
        for dc in range(nch):
            base = dc * S + qq * QTR
            nc.sync.dma_start(
                out=xt[:, dc, :], in_=io[nm][base : base + QTR, :],
                transpose=True,
            )
        return xt

    def proj_qk_one(xt, qq, wi, bias, dst, h, sqq):
        # scores-pool slot (not aux): aux holds the live pv accumulator, so a
        # second rotating tenant there would serialize every projection
        # against its DVE bias-add read
        sq = qq * SQQ + sqq
        ps = scores_pool.tile([128, SQ], F32, tag="scores", name=f"ps_{wi}_{qq}_{h}_{sqq}")
        if fp8_for(wi):
            # fp8 DoubleRow: 3 contraction chunks of 256 d (pairs d=256c+2ki+j
            # matching the byte-packed transpose and the host w8 layout);
            # each chunk streams N=512 at 0.5 cycles/row
            x8 = xt[:].bitcast(F8).rearrange("p c (s j) -> p c j s", j=2)
            for c in range(3):
                nc.tensor.matmul(
                    ps[:],
                    lhsT=w8[:, c, wi, :, h, :],
                    rhs=x8[:, c, :, sqq * SQ : (sqq + 1) * SQ],
                    start=(c == 0),
                    stop=(c == 2),
                    perf_mode=mybir.MatmulPerfMode.DoubleRow,
                )
            # undo the x32 weight scale, then add bias
            nc.vector.tensor_scalar(
                dst[:, h, sq * SQ : (sq + 1) * SQ], ps[:],
                0.03125, bias[:, h : h + 1], ALU.mult, ALU.add,
            )
        else:
            for dc in range(NDC):
                nc.tensor.matmul(
                    ps[:],
                    lhsT=w_dup[:, dc, wi, h, :],
                    rhs=xt[:, dc, sqq * SQ : (sqq + 1) * SQ],
                    start=(dc == 0),
                    stop=(dc == NDC - 1),
                )
            nc.vector.tensor_scalar(
                dst[:, h, sq * SQ : (sq + 1) * SQ], ps[:],
                bias[:, h : h + 1], None, ALU.add,
            )

    def proj_qk(xt, qq, wi, bias, dst, skip=None):
        for h in range(HPG):
            for sqq in range(SQQ):
                if skip is not None and (h, sqq) in skip:
                    continue
                proj_qk_one(xt, qq, wi, bias, dst, h, sqq)

    def proj_v(xt, qq, scqs=None):
        for scq in (range(SCQ) if scqs is None else scqs):
            sc = qq * SCQ + scq
            ps = scores_pool.tile([128, GD], F32, tag="scores", name=f"psv_{qq}_{scq}")
            for dc in range(NDC):
                nc.tensor.matmul(
                    ps[:],
                    lhsT=xt[:, dc, scq * 128 : (scq + 1) * 128],
                    rhs=wv_sb[:, dc, :],
                    start=(dc == 0),
                    stop=False,
                )
            nc.tensor.matmul(
                ps[:], lhsT=ones_row[:, 0:128], rhs=bv_sb[:], start=False, stop=True
            )
            for h in range(HPG):
                nc.vector.tensor_copy(
                    vE[:, sc, h, 0:DK], ps[:, h * DK : (h + 1) * DK]
                )
            # fold mask scale into v and the denominator ones column
            nc.vector.tensor_scalar(
                vE[:, sc, :, :], vE[:, sc, :, :], em_sb[:, sc : sc + 1], None,
                ALU.mult,
            )

    def proj_q_group(xt, qq, h, sqq):
        proj_qk_one(xt, qq, 0, bqbk[:, 0:HPG], qT, h, sqq)

    # ---- attention ----
    groups = []
    g0 = 0
    while g0 < NKC:
        groups.append((g0, min(GRP, NKC - g0)))
        g0 += GRP

    # ScalarE exp is the kernel's critical engine in steady state. Offload
    # the LAST `DVE_NGROUPS` exp groups of each iteration (latest PV
    # deadlines) to a DVE polynomial: exp(s/8) = h(y)^4 with y = s/32 and
    # h a degree-2 least-squares fit of exp on y in [-0.5, 0.5]. Squaring
    # twice keeps probs nonnegative by construction. 6 DVE ops per group
    # (1 PSUM-read + 5 fp16-SBUF ops) vs 1 ACT op; worth it because DVE is
    # ~80% idle while ACT is the wall. Adds ~3.7e-3 relative error on the
    # offloaded 5/32 of keys (budget 2e-2).
    # A/B on HW (same-process, interleaved): offload=2 groups measured ~26us
    # SLOWER (579 vs 553) despite ACT being the busiest engine — the DVE
    # chain latency sits on the iteration-transition critical path. Off by
    # default.
    DVE_NGROUPS = int(__import__("os").environ.get("BASS_DVE_NGROUPS", "0"))
    C0, C1, C2 = 1.00148143, 1.02379966, 0.48757841

    def emit_dve_exp(sc, pr, w, it, gi):
        yt = dx_pool.tile([128, w], BF16, tag="dx", name=f"y_{it}_{gi}")
        nc.vector.tensor_scalar(yt[:], sc[:], 0.03125, None, ALU.mult)
        t1 = dh_pool.tile([128, w], BF16, tag="dh", name=f"t1_{it}_{gi}")
        nc.vector.tensor_scalar(t1[:], yt[:], C2, C1, ALU.mult, ALU.add)
        t2 = dh_pool.tile([128, w], BF16, tag="dh", name=f"t2_{it}_{gi}")
        nc.vector.tensor_tensor(t2[:], t1[:], yt[:], ALU.mult)
        t3 = dh_pool.tile([128, w], BF16, tag="dh", name=f"t3_{it}_{gi}")
        nc.vector.tensor_scalar(t3[:], t2[:], C0, None, ALU.add)
        t4 = dh_pool.tile([128, w], BF16, tag="dh", name=f"t4_{it}_{gi}")
        nc.vector.tensor_tensor(t4[:], t3[:], t3[:], ALU.mult)
        nc.vector.tensor_tensor(pr[:], t4[:], t4[:], ALU.mult)

    pending = None  # finalize closure for the previous (h, sq)

    # finalize split in two: part 1 (the pv->SBUF copy, which frees the pv
    # PSUM slot) fires at gi==0 so it lands in the DVE queue BEFORE the
    # ~10us exp-poly chain; part 2 (PE transposes + normalize + store) at
    # gi==1 so the transposes sit behind QK g1 in the PE queue and never
    # stall on the copy.
    def make_fin_copy(pv, h, sq):
        ot = outt_pool.tile([DK + 1, SQ], F32, tag="outt", name=f"ot_{h}_{sq}")

        def fin1():
            nc.vector.tensor_copy(ot[:], pv[:])
        return ot, fin1

    def make_finalize(ot, h, sq):
        def fin():
            tr = aux_psum.tile([128, 4 * (DK + 1)], F32, tag="aux", name=f"tr_{h}_{sq}")
            for t in range(4):
                nc.tensor.transpose(
                    tr[:, t * (DK + 1) : (t + 1) * (DK + 1)],
                    ot[:, t * 128 : (t + 1) * 128],
                    ident[0 : DK + 1, 0 : DK + 1],
                )
            rc = small.tile([128, 4], F32, tag="recip", name=f"rc_{h}_{sq}")
            osl = oslab_pool.tile([128, 4, DK], F32, tag="oslab", name=f"os_{h}_{sq}")
            for t in range(4):
                nc.vector.reciprocal(
                    rc[:, t : t + 1], tr[:, t * (DK + 1) + DK : t * (DK + 1) + DK + 1]
                )
                nc.vector.tensor_scalar(
                    osl[:, t, :],
                    tr[:, t * (DK + 1) : t * (DK + 1) + DK],
                    rc[:, t : t + 1],
                    None,
                    ALU.mult,
                )
            nc.gpsimd.dma_start(
                out=io["out"].rearrange(
                    "(sq t p) n -> sq p t n", sq=NSQ, t=4, p=128
                )[sq, :, :, h * DK : (h + 1) * DK],
                in_=osl[:],
            )
        return fin

    # Boundary tasks: kept as an (empty by default) hook used by the carry
    # emission point inside attention_gen.
    boundary_tasks = {}
    xt_q_tiles = {}

    def attention_gen():
        nonlocal_pending = [None]

        def emit_pv(pv, h, grp, pvst):
            # start/stop are positional (pvst counts PV matmuls emitted for
            # this accumulator): chunk emission order is permuted when DVE
            # exp groups are deferred, so kc == 0 is not necessarily first.
            p0, plen, ppr = grp
            for j in range(plen):
                kc = p0 + j
                nc.tensor.matmul(
                    pv[:],
                    lhsT=vE[:, kc, h, :],
                    rhs=ppr[:, j * SQ : (j + 1) * SQ],
                    start=(pvst[0] == 0),
                    stop=(pvst[0] == NKC - 1),
                )
                pvst[0] += 1

        carry = None  # (pv, h, [groups]) tail-PV work carried across iterations
        it = 0
        ITER_HMINOR = __import__("os").environ.get("BASS_HMINOR", "1") == "1"
        # h-minor / sq-major order: qT quarter q is first needed at iteration
        # 3*2q, so the xq DMAs and qT projections can trail far behind the
        # kT/vE pipeline instead of gating it.
        if ITER_HMINOR:
            iters = [(h, sq) for sq in range(NSQ) for h in range(HPG)]
        else:
            iters = [(h, sq) for h in range(HPG) for sq in range(NSQ)]
        for h, sq in iters:
            # From iteration 2 on, the DVE-offloaded groups (the LAST k-chunk
            # groups) are FIRED FIRST: their scores slots free early (from the
            # previous iteration), and their ~5us DVE poly chains overlap this
            # iteration's ACT burst instead of delaying the carry PVs.
            offload = DVE_NGROUPS if it >= 2 else 0
            if offload:
                act_groups = groups[-offload:] + groups[:-offload]
            else:
                act_groups = groups
            dve_set = set(range(offload))
            pv = aux_psum.tile([DK + 1, SQ], F32, tag="aux", name=f"pv_{h}_{sq}")
            pvst = [0]
            ready = []  # (kc0, glen, probs) groups awaiting PV emission
            defer = []  # (eligible_fire_idx, grp) DVE groups awaiting poly
            # QK matmuls are emitted in strict (even, odd) kc pairs ACROSS
            # group boundaries so every matmul lands adjacent to its
            # opposite-row-group partner in the PE queue and the two K=64
            # halves run concurrently (PV blocks between groups would
            # otherwise orphan each group's 3rd chunk)
            chunk_list = []
            for gi, (kc0, glen) in enumerate(act_groups):
                for j in range(glen):
                    chunk_list.append((kc0 + j, gi, j))
            sc_tiles = {}
            filled = [0] * len(act_groups)
            fired = 0
            ci = 0
            # no pairing in iteration 0: its one-chunk lookahead would hold a
            # live scores tile across the prologue's advance() points, where
            # proj_q_group borrows slots from the same pool
            # A/B on HW: pairing measured neutral-to-worse (478us vs 450us
            # best-valid samples) — likely the 1-chunk lookahead couples the
            # PE to the previous exp via the scores double-buffer. Opt-in.
            do_pair = __import__("os").environ.get("BASS_PAIR", "0") == "1"
            pair_n = 2 if (it > 0 and do_pair) else 1
            while ci < len(chunk_list):
                for _ in range(pair_n):
                    if ci >= len(chunk_list):
                        break
                    kc, gi, j = chunk_list[ci]
                    ci += 1
                    if gi not in sc_tiles:
                        sc_tiles[gi] = scores_pool.tile(
                            [128, act_groups[gi][1] * SQ], F32, tag="scores",
                            name=f"sc_{h}_{sq}_{gi}",
                        )
                    ho = 64 * (kc % 2)
                    nc.tensor.matmul(
                        sc_tiles[gi][:, j * SQ : (j + 1) * SQ],
                        lhsT=kT[ho : ho + DK, h, kc * KCW : (kc + 1) * KCW],
                        rhs=qT[ho : ho + DK, h, sq * SQ : (sq + 1) * SQ],
                        start=True,
                        stop=True,
                    )
                    filled[gi] += 1
                while fired < len(act_groups) and (
                    filled[fired] == act_groups[fired][1]
                ):
                    gi = fired
                    kc0, glen = act_groups[gi]
                    pr = probs_pool.tile(
                        [128, glen * SQ], BF16, tag="probs",
                        name=f"pr_{h}_{sq}_{gi}",
                    )
                    if gi in dve_set:
                        emit_dve_exp(
                            sc_tiles.pop(gi), pr, glen * SQ, it, gi
                        )
                        defer.append((gi + 4, (kc0, glen, pr)))
                    else:
                        nc.scalar.activation(
                            pr[:], sc_tiles.pop(gi)[:], AF.Exp, scale=0.125
                        )
                        ready.append((kc0, glen, pr))
                    while defer and defer[0][0] <= gi:
                        ready.append(defer.pop(0)[1])
                    if gi == 0:
                        if carry is not None:
                            cpv, ch, cgrps, cpvst = carry
                            for grp in cgrps:
                                emit_pv(cpv, ch, grp, cpvst)
                            carry = None
                            for fn in boundary_tasks.get(it - 1, ()):
                                fn()
                        # free the previous pv PSUM slot (DVE copy) BEFORE
                        # the iteration's poly chains enter the DVE queue
                        if nonlocal_pending[0] is not None:
                            nonlocal_pending[0][0]()
                    if gi == 1 and nonlocal_pending[0] is not None:
                        nonlocal_pending[0][1]()
                        nonlocal_pending[0] = None
                    # iteration 0 holds one extra group in flight: its vE
                    # quarters are still streaming in, so the deeper lag keeps
                    # PV emission behind the xv DMA wavefront
                    if len(ready) >= (3 if it == 0 else 2):
                        emit_pv(pv, h, ready.pop(0), pvst)
                    fired += 1
                    yield (h, sq, gi)
            carry_grps = list(ready) + [g for _, g in defer]
            carry = (pv, h, carry_grps, pvst)
            ot, fin1 = make_fin_copy(pv, h, sq)
            nonlocal_pending[0] = (fin1, make_finalize(ot, h, sq))
            it += 1

        cpv, ch, cgrps, cpvst = carry
        for grp in cgrps:
            emit_pv(cpv, ch, grp, cpvst)
        nonlocal_pending[0][0]()
        nonlocal_pending[0][1]()

    # All x DMA-transposes are issued upfront in availability order so the
    # DMA engines stream continuously from t=0 (the quarter-chasing scheme
    # left them ~50% idle over a 100us window). kT quarters land first (they
    # gate iteration 0's exp groups), vE quarters interleave (they gate the
    # trailing PV), and xq quarters come last (h-minor iteration order defers
    # the first sq>=2 iteration far enough that qT projection can trail).
    gen = attention_gen()

    def advance(n):
        for _ in range(n):
            if next(gen, None) is None:
                break

    bk = bqbk[:, HPG : 2 * HPG]
    xt_k0 = load_xt_quarter("xk", 0)
    xt_q_tiles[0] = load_xt_quarter("xq", 0)
    xt_v0 = load_xt_quarter("xv", 0)
    xt_k1 = load_xt_quarter("xk", 1)
    xt_v1 = load_xt_quarter("xv", 1)
    xt_k2 = load_xt_quarter("xk", 2)
    xt_v2 = load_xt_quarter("xv", 2)
    xt_k3 = load_xt_quarter("xk", 3)
    # (xv3 + xq1..3 are issued below once xt slots have been consumed)

    # Fine-grained prologue: projections are emitted in DMA-availability
    # order, spread across iteration 0's fires so the PE never holds a long
    # burst ahead of runnable attention work, and per-(head, sqq) so only
    # head 0's kT gates iteration 0 (h1/h2 trail for iterations 1-2; all xt
    # readers still finish within iteration 0's span, freeing slots for
    # xv3/xq1-3).
    # PV(g) of iteration 0 is emitted at fire g+3 (pop threshold 3), so vE
    # chunks 3g..3g+2 must be emitted before fire g+3.
    proj_qk_one(xt_k0, 0, 1, bk, kT, 0, 0)
    proj_q_group(xt_q_tiles[0], 0, 0, 0)
    advance(1)   # f1: g0 (chunks 0-2)
    proj_qk_one(xt_k0, 0, 1, bk, kT, 0, 1)
    proj_qk_one(xt_k0, 0, 1, bk, kT, 1, 0)
    advance(1)   # f2: g1
    proj_v(xt_v0, 0, (0, 1, 2))
    proj_qk_one(xt_k1, 1, 1, bk, kT, 0, 0)    # g2's chunk 8
    advance(1)   # f3: g2 + PV(g0)
    proj_v(xt_v0, 0, (3, 4, 5))
    proj_qk_one(xt_k0, 0, 1, bk, kT, 1, 1)
    advance(1)   # f4: g3 + PV(g1)
    proj_v(xt_v0, 0, (6, 7))
    proj_v(xt_v1, 1, (0,))
    proj_qk_one(xt_k1, 1, 1, bk, kT, 0, 1)    # g4's chunks 12-14
    proj_qk_one(xt_k0, 0, 1, bk, kT, 2, 0)
    advance(1)   # f5: g4 + PV(g2)
    proj_v(xt_v1, 1, (1, 2, 3))
    proj_qk_one(xt_k2, 2, 1, bk, kT, 0, 0)    # g5's chunks 16-17
    proj_qk_one(xt_k0, 0, 1, bk, kT, 2, 1)
    advance(1)   # f6: g5 + PV(g3)
    proj_v(xt_v1, 1, (4, 5, 6))
    proj_qk_one(xt_k2, 2, 1, bk, kT, 0, 1)    # g6's chunk 20
    proj_qk_one(xt_k1, 1, 1, bk, kT, 1, 0)
    advance(1)   # f7: g6 + PV(g4)
    xt_v3 = load_xt_quarter("xv", 3)
    proj_v(xt_v1, 1, (7,))
    proj_v(xt_v2, 2, (0, 1))
    proj_qk_one(xt_k1, 1, 1, bk, kT, 1, 1)
    advance(1)   # f8: g7 + PV(g5)
    proj_v(xt_v2, 2, (2, 3, 4))
    proj_qk_one(xt_k3, 3, 1, bk, kT, 0, 0)    # g8's chunk 24
    proj_qk_one(xt_k1, 1, 1, bk, kT, 2, 0)
    advance(1)   # f9: g8 + PV(g6)
    proj_v(xt_v2, 2, (5, 6, 7))
    proj_qk_one(xt_k3, 3, 1, bk, kT, 0, 1)    # g9's chunk 28
    proj_qk_one(xt_k1, 1, 1, bk, kT, 2, 1)
    xt_q_tiles[1] = load_xt_quarter("xq", 1)
    advance(1)   # f10: g9 + PV(g7)
    proj_v(xt_v3, 3, (0, 1, 2))
    proj_qk_one(xt_k2, 2, 1, bk, kT, 1, 0)
    proj_qk_one(xt_k2, 2, 1, bk, kT, 1, 1)
    xt_q_tiles[2] = load_xt_quarter("xq", 2)
    advance(1)   # f11: g10 + PV(g8) — iteration 0 groups complete
    proj_v(xt_v3, 3, (3, 4, 5, 6, 7))
    proj_q_group(xt_q_tiles[0], 0, 1, 0)      # iter1 = (h1, s0) at f12
    proj_qk_one(xt_k2, 2, 1, bk, kT, 2, 0)
    proj_qk_one(xt_k2, 2, 1, bk, kT, 2, 1)
    xt_q_tiles[3] = load_xt_quarter("xq", 3)
    advance(2)   # f12 (iter1 g0: carry PVs g9,g10 + fin), f13
    proj_qk_one(xt_k3, 3, 1, bk, kT, 1, 0)    # iter1 g8 reads at f20
    proj_qk_one(xt_k3, 3, 1, bk, kT, 1, 1)
    advance(2)   # f14, f15
    proj_qk_one(xt_k3, 3, 1, bk, kT, 2, 0)    # iter2 g8 reads at f31
    proj_qk_one(xt_k3, 3, 1, bk, kT, 2, 1)
    proj_q_group(xt_q_tiles[0], 0, 2, 0)      # iter2 = (h2, s0) at f23
    advance(7)   # f16-f22
    proj_q_group(xt_q_tiles[0], 0, 0, 1)      # iter3 = (h0, s1) at f34
    advance(2)   # f23, f24
    proj_q_group(xt_q_tiles[0], 0, 1, 1)      # iter4 at f45
    advance(5)   # f25-f29
    proj_q_group(xt_q_tiles[0], 0, 2, 1)      # iter5 at f56
    advance(5)   # f30-f34
    # qT quarters 1-3: one projection per 5 fires from ~f35 (deadline for
    # quarter q head h is fire 11*(6q+h), loose for every entry)
    for qq in (1, 2, 3):
        for h in range(HPG):
            for sqq in range(SQQ):
                proj_q_group(xt_q_tiles[qq], qq, h, sqq)
                advance(5)
    for _ in gen:
        pass


def _build():
    nc = bacc.Bacc("TRN2", target_bir_lowering=False, debug=False)
    mode = os.environ.get("BASS_FP8QK", "k")
    io = {}
    tensors = [
        ("xv", [NDC * S, 128], BF16), ("wv", [D, GD], BF16),
        ("bqbk_pk", [128, 2 * HPG], F32),
        ("bv_r", [1, GD], BF16), ("mask_pk", [128, NKC], F32),
    ]
    if mode in ("1", "k"):
        tensors.append(("wqk8", [128, 3 * 2 * 2 * HPG * 128], mybir.dt.float8e4))
        tensors.append(("xk", [3 * S, 128], BF16))
    else:
        tensors += [("xk", [NDC * S, 128], BF16), ("wk", [D, GD], BF16)]
    if mode == "1":
        tensors.append(("xq", [3 * S, 128], BF16))
    else:
        tensors += [("xq", [NDC * S, 128], BF16), ("wq", [D, GD], BF16)]
    for nm, shape, dt in tensors:
        io[nm] = nc.dram_tensor(nm, shape, dt, kind="ExternalInput").ap()
    io["out"] = nc.dram_tensor("out", [S, GD], F32, kind="ExternalOutput").ap()

    dup = int(os.environ.get("BASS_DUP", "1"))
    with tile.TileContext(nc) as tc:
        for _ in range(dup):
            with ExitStack() as ctx:
                _emit(ctx, tc, io)
    nc.compile()
    return nc


_NC = None


def _get_nc():
    global _NC
    if _NC is None:
        _NC = _build()
    return _NC


F8_NP = None


def _f8np():
    global F8_NP
    if F8_NP is None:
        from concourse import mybir as _mb
        F8_NP = _mb.dt.np(_mb.dt.float8e4)
    return F8_NP


def _pack_x8(a):
    # [S, D] f32 -> fp8 -> byte-pair uint16 carrier, d-pair-chunk-major
    # [3*S, 128] viewed as fp16 for the 2-byte xbar transpose
    a8 = np.ascontiguousarray(np.asarray(a, np.float32)).astype(_f8np())
    u = a8.view(np.uint8).reshape(S, 384, 2).view(np.uint16).reshape(S, 3, 128)
    return np.ascontiguousarray(
        u.transpose(1, 0, 2).reshape(3 * S, 128)
    ).view(np.float16)


def _pack_wqk8(Wq_c, Wk_c):
    # (ki, c, i, j, h, m) = 32 * W_i[256c + 2ki + j, h*64 + m%64]
    out = np.empty((128, 3, 2, 2, HPG, 128), np.float32)
    for i, W in ((0, Wq_c), (1, Wk_c)):
        Wr = (np.asarray(W, np.float32) * 32.0).reshape(3, 128, 2, HPG, DK)
        Wm = np.concatenate([Wr, Wr], axis=-1)        # [c, ki, j, h, 128]
        out[:, :, i] = Wm.transpose(1, 0, 2, 3, 4)    # [ki, c, j, h, 128]
    return np.ascontiguousarray(out.reshape(128, -1)).astype(_f8np())


def make_in_maps(query, key, value, mask, Wq, bq, Wk, bk, Wv, bv):
    mode = os.environ.get("BASS_FP8QK", "k")
    bf = lambda a: np.ascontiguousarray(a).astype(BF16_NP)
    bf3 = lambda a: np.ascontiguousarray(
        np.asarray(a).reshape(S, NDC, 128).transpose(1, 0, 2).reshape(NDC * S, 128)
    ).astype(BF16_NP)
    f32 = lambda a: np.ascontiguousarray(np.asarray(a, np.float32))
    in_maps = []
    for c in range(N_CORES):
        b, g = divmod(c, 4)
        cols = slice(g * GD, (g + 1) * GD)
        m = {
            "xv": bf3(value[b]),
            "wv": bf(Wv[:, cols]),
            "bqbk_pk": f32(np.tile(np.concatenate(
                [np.asarray(bq)[cols].reshape(HPG, DK).T,
                 np.asarray(bk)[cols].reshape(HPG, DK).T], axis=1), (2, 1))),
            "bv_r": bf(np.asarray(bv)[cols].reshape(1, GD)),
            "mask_pk": f32(np.asarray(mask)[b].reshape(NKC, 128).T),
        }
        if mode in ("1", "k"):
            m["xk"] = _pack_x8(key[b])
            m["wqk8"] = _pack_wqk8(
                np.asarray(Wq)[:, cols], np.asarray(Wk)[:, cols]
            )
        else:
            m["xk"] = bf3(key[b])
            m["wk"] = bf(Wk[:, cols])
        if mode == "1":
            m["xq"] = _pack_x8(query[b])
        else:
            m["xq"] = bf3(query[b])
            m["wq"] = bf(Wq[:, cols])
        in_maps.append(m)
    return in_maps


def kernel(query, key, value, mask, Wq, bq, Wk, bk, Wv, bv):
    query = np.asarray(query, np.float32)
    key = np.asarray(key, np.float32)
    value = np.asarray(value, np.float32)
    nc = _get_nc()
    in_maps = make_in_maps(query, key, value, mask, Wq, bq, Wk, bk, Wv, bv)
    res = run_bass_kernel_spmd(nc, in_maps, core_ids=list(range(N_CORES)))
    out = np.empty((B, S, D), np.float32)
    for c in range(N_CORES):
        b, g = divmod(c, 4)
        out[b, :, g * GD : (g + 1) * GD] = res.results[c]["out"]
    return out



# revision 52
# speedup vs baseline: 1.1489x; 1.0434x over previous
"""Multi-head attention (B=2, S=4096, D=768, H=12) on 8 Trainium2 cores.

Sharding: core c -> batch b = c // 4, head-triple g = c % 4 (heads 3g..3g+2).
Each core computes its QKV projections (columns of W for its heads) and
flash-style attention for its 3 heads, fully on-chip; no cross-core comms.
Host-side prep per core: slice batch/head-group, cast x/W to fp16 (f32
accumulation on device; fp16 over bf16 because all value ranges here are
tiny, ~8x lower quantization error at identical PE throughput). The K
projection inputs additionally go to fp8 (see below).

Per-core device kernel:
  - x^T tiles via xbar DMA-transpose straight from DRAM; ALL transposes are
    issued upfront in availability order (copies first on the same SP queue
    — mixed-queue issue interleaves copy/transpose at the shared HWDGE and
    every xbar-mode flip costs a drain), so the DMA engines stream
    continuously: kT quarters first, vE interleaved, xq last.
  - projections on PE produce qT/kT [64, 3, 4096] fp16 (duplicated on both
    partition halves) and v_ext [128, 32, 3, 65] (col 64 = ones so the PV
    matmul accumulates the softmax denominator as output row 64). The mask
    enters as a per-k scale em = exp(-1e4*(1-mask)) folded into v_ext
    (exact, including the denominator).
  - K projection (BASS_FP8QK="k", default) runs in fp8e4m3 DoubleRow: the
    host packs adjacent-d pairs of fp8 x into uint16 so the 2-byte xbar
    transpose yields the [128, 2, s] pair layout, and W*32 (lifted out of
    fp8's subnormal range; un-scaled in the bias-add) packed to match.
    Halves xk DMA bytes and K-proj PE streaming. Measured end-to-end err
    1.27e-2 l2 / 1.61e-2 absmax vs the 2e-2 gate. "1" extends it to Q
    (another -11us, but err 1.80e-2 l2 / 2.4e-2 absmax — too close).
  - attention iterations (h, sq) in h-minor order (sq-major), so qT quarter
    q is first needed at iteration 6q and the xq DMAs + qT projections trail
    far behind the kT/vE pipeline. Per iteration: 32 k-chunks in groups of
    3; QK^T -> PSUM, exp on ScalarE (scale=1/8) -> fp16 probs in SBUF, PV
    accumulate -> PSUM [65, 512] (positional start/stop flags); then
    PE-transpose and DVE normalize by the reciprocal of the denominator row.
  - prologue: projections are emitted fine-grained (per head/sqq; vE in
    chunk triplets) interleaved with iteration 0's exp groups, tracking DMA
    arrival; only (h0,s0)'s k+q projections gate the first exp. Iteration
    0 uses PV pop-threshold 3 (vE still streaming); trailing qT projections
    spread one per 5 exp groups through iterations ~2-10.

Perf notes (HW-verified this series):
  - QK^T row tiling: odd k-chunks read the qT/kT replicas on partitions
    64-127 so consecutive K=64 matmuls run on different PE row groups
    (historically 589us -> 388us); reason for the column duplication.
  - split finalize: the pv->SBUF copy is emitted at the NEXT iteration's
    first exp group so the pv PSUM slot frees early; transposes+normalize
    one group later so they queue behind QK g1 on the PE.
  - projection PSUM tiles borrow scores-pool slots (NOT aux): aux holds the
    live pv accumulator, and a second rotating aux tenant serializes every
    projection against its DVE bias-add.
  - rejected on same-process HW A/B: DVE exp offload (BASS_DVE_NGROUPS=2,
    deg-2+double-squaring poly on the last 2 groups/iter) measured 579 vs
    553us — the DVE chain latency lands on the iteration transition.
    BASS_PAIR=1 (QK emission pairing across group boundaries) also negative.
  - PSUM budget: scores 2 bufs x 3 banks + pv 1 + tr 1 = 8 (full).
  - measurement: cross-process HW timing drifts ~+-8%; only same-process
    interleaved A/B slopes (ab.py / ab2.py) are trustworthy.
"""

import os
import sys

if "/opt/trn_rl_repo" not in sys.path:
    sys.path.insert(0, "/opt/trn_rl_repo")

from contextlib import ExitStack

import ml_dtypes
import numpy as np

import concourse.bass as bass
import concourse.tile as tile
from concourse import bacc, mybir
from concourse.bass_utils import run_bass_kernel_spmd
from concourse.masks import make_identity

F32 = mybir.dt.float32
# fp16 instead of bf16: all on-chip value ranges here are tiny (|x|<6,
# |W|<0.12, probs<8), so fp16's 10 mantissa bits cut quantization error ~4x
# at identical PE throughput (1 cycle/row) and xbar 2-byte transpose support
BF16 = mybir.dt.float16
AF = mybir.ActivationFunctionType
ALU = mybir.AluOpType
BF16_NP = np.float16

B, S, D, H, DK = 2, 4096, 768, 12, 64
N_CORES = 8
HPG = 3            # heads per core
GD = HPG * DK      # 192 output columns per core
SQ = 512           # q-chunk width
NSQ = S // SQ      # 8
KCW = 128          # k-chunk width
NKC = S // KCW     # 32
GRP = 3            # k-chunks per exp group (3 PSUM banks, double buffered)
NDC = D // 128     # 6 contraction chunks
QTR = S // 4       # transpose/projection pipeline granularity
SQQ = NSQ // 4     # q chunks per quarter
SCQ = NKC // 4     # s chunks per quarter


def _emit(ctx: ExitStack, tc: tile.TileContext, io: dict):
    nc = tc.nc

    const = ctx.enter_context(tc.tile_pool(name="const", bufs=1))
    # 8 slots: quarters xk0-3/xq0/xv0-2 are all live early; xv3 (9th alloc)
    # then reuses xk0's slot, whose readers finish by ~f6 — reuse of any
    # later slot (e.g. xq0's, read until ~f30) would deadlock the PE queue
    # against iteration 0's PV(g8).
    xt_pool = ctx.enter_context(tc.tile_pool(name="xt", bufs=8))
    proj = ctx.enter_context(tc.tile_pool(name="proj", bufs=1))
    scores_pool = ctx.enter_context(tc.tile_pool(name="scores", bufs=2, space="PSUM"))
    aux_psum = ctx.enter_context(tc.tile_pool(name="auxp", bufs=2, space="PSUM"))
    probs_pool = ctx.enter_context(tc.tile_pool(name="probs", bufs=5))
    outt_pool = ctx.enter_context(tc.tile_pool(name="outt", bufs=2))
    small = ctx.enter_context(tc.tile_pool(name="small", bufs=2))
    oslab_pool = ctx.enter_context(tc.tile_pool(name="oslab", bufs=3))
    # DVE-exp offload scratch: x tile, Horner ping-pong, and probs output
    dx_pool = ctx.enter_context(tc.tile_pool(name="dx", bufs=2))
    dh_pool = ctx.enter_context(tc.tile_pool(name="dh", bufs=3))
    prd_pool = ctx.enter_context(tc.tile_pool(name="prd", bufs=2))

    # ---- constants / small inputs (consolidated to limit 4KB slot padding) ----
    # mask -> per-k scale em = exp(-1e4 * (1 - mask)), [128, 32] (p, kchunk).
    # Emitted FIRST so the ACT exp-table load lands at the head of the queues.
    # All const DMAs go on the SAME queue (SP) that later issues the x
    # DMA-transposes: the shared HWDGE serializes globally and every
    # copy<->transpose transition costs an xbar-mode drain, so mixed-queue
    # issue (copies from ACT, transposes from SP) interleaved them worst-case.
    mask_em = const.tile([128, 65], F32, name="mask_em")
    mask_t = mask_em[:, 0:32]
    em_sb = mask_em[:, 32:64]
    neg1e4 = mask_em[:, 64:65]
    nc.gpsimd.memset(neg1e4, -10000.0)
    nc.sync.dma_start(mask_t, io["mask_pk"][:])
    nc.scalar.activation(em_sb, mask_t, AF.Exp, scale=10000.0, bias=neg1e4)

    # "0": all-fp16 projections. "1": q AND k projections via fp8 DoubleRow
    # (hw-measured end-to-end rel err 1.8e-2 — too close to the 2e-2 gate).
    # "k": only the K projection in fp8 (err ~1.2e-2 l2 / 1.5e-2 absmax,
    # comfortable margin) at half the PE savings.
    FP8MODE = __import__("os").environ.get("BASS_FP8QK", "k")
    FP8QK = FP8MODE in ("1", "k")
    F8 = mybir.dt.float8e4

    def fp8_for(wi):
        return FP8MODE == "1" or (FP8MODE == "k" and wi == 1)

    if FP8QK:
        # q/k weights as fp8 DoubleRow pairs, host-packed in tile layout:
        # (ki, c, i, j, h, m) = 32*W_i[256c + 2ki + j, h*64 + m%64]
        # (x32 lifts W sigma=0.02 out of fp8's subnormal range; the bias-add
        # multiplies the PSUM result by 1/32)
        w8 = const.tile([128, 3, 2, 2, HPG, 128], F8, name="w8")
        nc.sync.dma_start(
            w8[:],
            io["wqk8"].rearrange(
                "p (c i j h m) -> p c i j h m", c=3, i=2, j=2, h=HPG
            ),
        )
    # fp16 weight slabs: v always; q and/or k when their projection is fp16
    fp16_w = [(2, "wv")]
    if not fp8_for(0):
        fp16_w.append((0, "wq"))
    if not fp8_for(1):
        fp16_w.append((1, "wk"))
    w_all = const.tile([128, NDC, 3 * GD], BF16, name="w_all")
    for i, nm in fp16_w:
        nc.sync.dma_start(
            w_all[:, :, i * GD : (i + 1) * GD],
            io[nm].rearrange("(dc p) n -> p dc n", p=128),
        )
    wv_sb = w_all[:, :, 2 * GD : 3 * GD]

    if not (fp8_for(0) and fp8_for(1)):
        # q/k weights with each head's 64 columns duplicated (projection then
        # replicates qT/kT on both partition halves at no extra PE cost)
        w_dup = const.tile([128, NDC, 2, HPG, 128], BF16, name="w_dup")
        for i, _nm in fp16_w:
            if i == 2:
                continue
            for h in range(HPG):
                for rep in range(2):
                    nc.vector.tensor_copy(
                        w_dup[:, :, i, h, rep * DK : (rep + 1) * DK],
                        w_all[:, :, i * GD + h * DK : i * GD + (h + 1) * DK],
                    )

    bqbk = const.tile([128, 2 * HPG], F32, name="bqbk")
    nc.sync.dma_start(bqbk[:], io["bqbk_pk"][:])

    bfpack = const.tile([1, 320], BF16, name="bfpack")
    nc.gpsimd.memset(bfpack[:, 0:128], 1.0)
    nc.sync.dma_start(bfpack[:, 128 : 128 + GD], io["bv_r"][:])
    ones_row = bfpack[:, 0:128]
    bv_sb = bfpack[:, 128 : 128 + GD]

    ident = const.tile([128, 128], F32, name="ident")
    make_identity(nc, ident[:])

    # ---- persistent projection outputs (qT/kT replicated on both halves) ----
    qT = proj.tile([128, HPG, S], BF16, name="qT")
    kT = proj.tile([128, HPG, S], BF16, name="kT")
    vE = proj.tile([128, NKC, HPG, DK + 1], BF16, name="vE")
    nc.gpsimd.memset(vE[:], 1.0)  # ones col 64; data cols overwritten below

    # ---- per-quarter: transpose + project ----
    def load_xt_quarter(nm, qq):
        # host supplies x d-chunk-major [6*4096, 128] so each xbar transpose
        # reads a fully contiguous [1024, 128] block. In FP8QK mode, xq/xk
        # arrive byte-packed (two fp8 d-neighbors per uint16 element): 3
        # chunks of 128 pair-columns, half the DMA bytes.
        packed = (nm == "xk" and FP8QK) or (nm == "xq" and FP8MODE == "1")
        nch = 3 if packed else NDC
        xt = xt_pool.tile([128, nch, QTR], BF16, tag="xt", name=f"xt_{nm}_{qq}")
        for dc in range(nch):
            base = dc * S + qq * QTR
            nc.sync.dma_start(
                out=xt[:, dc, :], in_=io[nm][base : base + QTR, :],
                transpose=True,
            )
        return xt

    def proj_qk_one(xt, qq, wi, bias, dst, h, sqq):
        # scores-pool slot (not aux): aux holds the live pv accumulator, so a
        # second rotating tenant there would serialize every projection
        # against its DVE bias-add read
        sq = qq * SQQ + sqq
        ps = scores_pool.tile([128, SQ], F32, tag="scores", name=f"ps_{wi}_{qq}_{h}_{sqq}")
        if fp8_for(wi):
            # fp8 DoubleRow: 3 contraction chunks of 256 d (pairs d=256c+2ki+j
            # matching the byte-packed transpose and the host w8 layout);
            # each chunk streams N=512 at 0.5 cycles/row
            x8 = xt[:].bitcast(F8).rearrange("p c (s j) -> p c j s", j=2)
            for c in range(3):
                nc.tensor.matmul(
                    ps[:],
                    lhsT=w8[:, c, wi, :, h, :],
                    rhs=x8[:, c, :, sqq * SQ : (sqq + 1) * SQ],
                    start=(c == 0),
                    stop=(c == 2),
                    perf_mode=mybir.MatmulPerfMode.DoubleRow,
                )
            # undo the x32 weight scale, then add bias
            nc.vector.tensor_scalar(
                dst[:, h, sq * SQ : (sq + 1) * SQ], ps[:],
                0.03125, bias[:, h : h + 1], ALU.mult, ALU.add,
            )
        else:
            for dc in range(NDC):
                nc.tensor.matmul(
                    ps[:],
                    lhsT=w_dup[:, dc, wi, h, :],
                    rhs=xt[:, dc, sqq * SQ : (sqq + 1) * SQ],
                    start=(dc == 0),
                    stop=(dc == NDC - 1),
                )
            nc.vector.tensor_scalar(
                dst[:, h, sq * SQ : (sq + 1) * SQ], ps[:],
                bias[:, h : h + 1], None, ALU.add,
            )

    def proj_qk(xt, qq, wi, bias, dst, skip=None):
        for h in range(HPG):
            for sqq in range(SQQ):
                if skip is not None and (h, sqq) in skip:
                    continue
                proj_qk_one(xt, qq, wi, bias, dst, h, sqq)

    def proj_v(xt, qq, scqs=None):
        for scq in (range(SCQ) if scqs is None else scqs):
            sc = qq * SCQ + scq
            ps = scores_pool.tile([128, GD], F32, tag="scores", name=f"psv_{qq}_{scq}")
            for dc in range(NDC):
                nc.tensor.matmul(
                    ps[:],
                    lhsT=xt[:, dc, scq * 128 : (scq + 1) * 128],
                    rhs=wv_sb[:, dc, :],
                    start=(dc == 0),
                    stop=False,
                )
            nc.tensor.matmul(
                ps[:], lhsT=ones_row[:, 0:128], rhs=bv_sb[:], start=False, stop=True
            )
            for h in range(HPG):
                nc.vector.tensor_copy(
                    vE[:, sc, h, 0:DK], ps[:, h * DK : (h + 1) * DK]
                )
            # fold mask scale into v and the denominator ones column
            nc.vector.tensor_scalar(
                vE[:, sc, :, :], vE[:, sc, :, :], em_sb[:, sc : sc + 1], None,
                ALU.mult,
            )

    def proj_q_group(xt, qq, h, sqq):
        proj_qk_one(xt, qq, 0, bqbk[:, 0:HPG], qT, h, sqq)

    # ---- attention ----
    groups = []
    g0 = 0
    while g0 < NKC:
        groups.append((g0, min(GRP, NKC - g0)))
        g0 += GRP

    # ScalarE exp is the kernel's critical engine in steady state. Offload
    # the LAST `DVE_NGROUPS` exp groups of each iteration (latest PV
    # deadlines) to a DVE polynomial: exp(s/8) = h(y)^4 with y = s/32 and
    # h a degree-2 least-squares fit of exp on y in [-0.5, 0.5]. Squaring
    # twice keeps probs nonnegative by construction. 6 DVE ops per group
    # (1 PSUM-read + 5 fp16-SBUF ops) vs 1 ACT op; worth it because DVE is
    # ~80% idle while ACT is the wall. Adds ~3.7e-3 relative error on the
    # offloaded 5/32 of keys (budget 2e-2).
    # A/B on HW (same-process, interleaved): offload=2 groups measured ~26us
    # SLOWER (579 vs 553) despite ACT being the busiest engine — the DVE
    # chain latency sits on the iteration-transition critical path. Off by
    # default.
    DVE_NGROUPS = int(__import__("os").environ.get("BASS_DVE_NGROUPS", "0"))
    C0, C1, C2 = 1.00148143, 1.02379966, 0.48757841

    def emit_dve_exp(sc, pr, w, it, gi):
        yt = dx_pool.tile([128, w], BF16, tag="dx", name=f"y_{it}_{gi}")
        nc.vector.tensor_scalar(yt[:], sc[:], 0.03125, None, ALU.mult)
        t1 = dh_pool.tile([128, w], BF16, tag="dh", name=f"t1_{it}_{gi}")
        nc.vector.tensor_scalar(t1[:], yt[:], C2, C1, ALU.mult, ALU.add)
        t2 = dh_pool.tile([128, w], BF16, tag="dh", name=f"t2_{it}_{gi}")
        nc.vector.tensor_tensor(t2[:], t1[:], yt[:], ALU.mult)
        t3 = dh_pool.tile([128, w], BF16, tag="dh", name=f"t3_{it}_{gi}")
        nc.vector.tensor_scalar(t3[:], t2[:], C0, None, ALU.add)
        t4 = dh_pool.tile([128, w], BF16, tag="dh", name=f"t4_{it}_{gi}")
        nc.vector.tensor_tensor(t4[:], t3[:], t3[:], ALU.mult)
        nc.vector.tensor_tensor(pr[:], t4[:], t4[:], ALU.mult)

    pending = None  # finalize closure for the previous (h, sq)

    # finalize split in two: part 1 (the pv->SBUF copy, which frees the pv
    # PSUM slot) fires at gi==0 so it lands in the DVE queue BEFORE the
    # ~10us exp-poly chain; part 2 (PE transposes + normalize + store) at
    # gi==1 so the transposes sit behind QK g1 in the PE queue and never
    # stall on the copy.
    def make_fin_copy(pv, h, sq):
        ot = outt_pool.tile([DK + 1, SQ], F32, tag="outt", name=f"ot_{h}_{sq}")

        def fin1():
            nc.vector.tensor_copy(ot[:], pv[:])
        return ot, fin1

    def make_finalize(ot, h, sq):
        def fin():
            tr = aux_psum.tile([128, 4 * (DK + 1)], F32, tag="aux", name=f"tr_{h}_{sq}")
            for t in range(4):
                nc.tensor.transpose(
                    tr[:, t * (DK + 1) : (t + 1) * (DK + 1)],
                    ot[:, t * 128 : (t + 1) * 128],
                    ident[0 : DK + 1, 0 : DK + 1],
                )
            rc = small.tile([128, 4], F32, tag="recip", name=f"rc_{h}_{sq}")
            osl = oslab_pool.tile([128, 4, DK], F32, tag="oslab", name=f"os_{h}_{sq}")
            for t in range(4):
                nc.vector.reciprocal(
                    rc[:, t : t + 1], tr[:, t * (DK + 1) + DK : t * (DK + 1) + DK + 1]
                )
                nc.vector.tensor_scalar(
                    osl[:, t, :],
                    tr[:, t * (DK + 1) : t * (DK + 1) + DK],
                    rc[:, t : t + 1],
                    None,
                    ALU.mult,
                )
            nc.gpsimd.dma_start(
                out=io["out"].rearrange(
                    "(sq t p) n -> sq p t n", sq=NSQ, t=4, p=128
                )[sq, :, :, h * DK : (h + 1) * DK],
                in_=osl[:],
            )
        return fin

    # Boundary tasks: kept as an (empty by default) hook used by the carry
    # emission point inside attention_gen.
    boundary_tasks = {}
    xt_q_tiles = {}

    def attention_gen():
        nonlocal_pending = [None]

        def emit_pv(pv, h, grp, pvst):
            # start/stop are positional (pvst counts PV matmuls emitted for
            # this accumulator): chunk emission order is permuted when DVE
            # exp groups are deferred, so kc == 0 is not necessarily first.
            p0, plen, ppr = grp
            for j in range(plen):
                kc = p0 + j
                nc.tensor.matmul(
                    pv[:],
                    lhsT=vE[:, kc, h, :],
                    rhs=ppr[:, j * SQ : (j + 1) * SQ],
                    start=(pvst[0] == 0),
                    stop=(pvst[0] == NKC - 1),
                )
                pvst[0] += 1

        carry = None  # (pv, h, [groups]) tail-PV work carried across iterations
        it = 0
        PVLAG3 = __import__("os").environ.get("BASS_PVLAG3", "0") == "1"
        ITER_HMINOR = __import__("os").environ.get("BASS_HMINOR", "1") == "1"
        # h-minor / sq-major order: qT quarter q is first needed at iteration
        # 3*2q, so the xq DMAs and qT projections can trail far behind the
        # kT/vE pipeline instead of gating it.
        if ITER_HMINOR:
            iters = [(h, sq) for sq in range(NSQ) for h in range(HPG)]
        else:
            iters = [(h, sq) for h in range(HPG) for sq in range(NSQ)]
        for h, sq in iters:
            # From iteration 2 on, the DVE-offloaded groups (the LAST k-chunk
            # groups) are FIRED FIRST: their scores slots free early (from the
            # previous iteration), and their ~5us DVE poly chains overlap this
            # iteration's ACT burst instead of delaying the carry PVs.
            offload = DVE_NGROUPS if it >= 2 else 0
            if offload:
                act_groups = groups[-offload:] + groups[:-offload]
            else:
                act_groups = groups
            dve_set = set(range(offload))
            pv = aux_psum.tile([DK + 1, SQ], F32, tag="aux", name=f"pv_{h}_{sq}")
            pvst = [0]
            ready = []  # (kc0, glen, probs) groups awaiting PV emission
            defer = []  # (eligible_fire_idx, grp) DVE groups awaiting poly
            # QK matmuls are emitted in strict (even, odd) kc pairs ACROSS
            # group boundaries so every matmul lands adjacent to its
            # opposite-row-group partner in the PE queue and the two K=64
            # halves run concurrently (PV blocks between groups would
            # otherwise orphan each group's 3rd chunk)
            chunk_list = []
            for gi, (kc0, glen) in enumerate(act_groups):
                for j in range(glen):
                    chunk_list.append((kc0 + j, gi, j))
            sc_tiles = {}
            filled = [0] * len(act_groups)
            fired = 0
            ci = 0
            # no pairing in iteration 0: its one-chunk lookahead would hold a
            # live scores tile across the prologue's advance() points, where
            # proj_q_group borrows slots from the same pool
            # A/B on HW: pairing measured neutral-to-worse (478us vs 450us
            # best-valid samples) — likely the 1-chunk lookahead couples the
            # PE to the previous exp via the scores double-buffer. Opt-in.
            do_pair = __import__("os").environ.get("BASS_PAIR", "0") == "1"
            pair_n = 2 if (it > 0 and do_pair) else 1
            while ci < len(chunk_list):
                for _ in range(pair_n):
                    if ci >= len(chunk_list):
                        break
                    kc, gi, j = chunk_list[ci]
                    ci += 1
                    if gi not in sc_tiles:
                        sc_tiles[gi] = scores_pool.tile(
                            [128, act_groups[gi][1] * SQ], F32, tag="scores",
                            name=f"sc_{h}_{sq}_{gi}",
                        )
                    ho = 64 * (kc % 2)
                    nc.tensor.matmul(
                        sc_tiles[gi][:, j * SQ : (j + 1) * SQ],
                        lhsT=kT[ho : ho + DK, h, kc * KCW : (kc + 1) * KCW],
                        rhs=qT[ho : ho + DK, h, sq * SQ : (sq + 1) * SQ],
                        start=True,
                        stop=True,
                    )
                    filled[gi] += 1
                while fired < len(act_groups) and (
                    filled[fired] == act_groups[fired][1]
                ):
                    gi = fired
                    kc0, glen = act_groups[gi]
                    pr = probs_pool.tile(
                        [128, glen * SQ], BF16, tag="probs",
                        name=f"pr_{h}_{sq}_{gi}",
                    )
                    if gi in dve_set:
                        emit_dve_exp(
                            sc_tiles.pop(gi), pr, glen * SQ, it, gi
                        )
                        defer.append((gi + 4, (kc0, glen, pr)))
                    else:
                        nc.scalar.activation(
                            pr[:], sc_tiles.pop(gi)[:], AF.Exp, scale=0.125
                        )
                        ready.append((kc0, glen, pr))
                    while defer and defer[0][0] <= gi:
                        ready.append(defer.pop(0)[1])
                    if gi == 0:
                        if carry is not None:
                            cpv, ch, cgrps, cpvst = carry
                            for grp in cgrps:
                                emit_pv(cpv, ch, grp, cpvst)
                            carry = None
                            for fn in boundary_tasks.get(it - 1, ()):
                                fn()
                        # free the previous pv PSUM slot (DVE copy) BEFORE
                        # the iteration's poly chains enter the DVE queue
                        if nonlocal_pending[0] is not None:
                            nonlocal_pending[0][0]()
                    if gi == 1 and nonlocal_pending[0] is not None:
                        nonlocal_pending[0][1]()
                        nonlocal_pending[0] = None
                    # iteration 0 holds one extra group in flight: its vE
                    # quarters are still streaming in, so the deeper lag keeps
                    # PV emission behind the xv DMA wavefront
                    if len(ready) >= (3 if (it == 0 or PVLAG3) else 2):
                        emit_pv(pv, h, ready.pop(0), pvst)
                    fired += 1
                    yield (h, sq, gi)
            carry_grps = list(ready) + [g for _, g in defer]
            carry = (pv, h, carry_grps, pvst)
            ot, fin1 = make_fin_copy(pv, h, sq)
            nonlocal_pending[0] = (fin1, make_finalize(ot, h, sq))
            it += 1

        cpv, ch, cgrps, cpvst = carry
        for grp in cgrps:
            emit_pv(cpv, ch, grp, cpvst)
        nonlocal_pending[0][0]()
        nonlocal_pending[0][1]()

    # All x DMA-transposes are issued upfront in availability order so the
    # DMA engines stream continuously from t=0 (the quarter-chasing scheme
    # left them ~50% idle over a 100us window). kT quarters land first (they
    # gate iteration 0's exp groups), vE quarters interleave (they gate the
    # trailing PV), and xq quarters come last (h-minor iteration order defers
    # the first sq>=2 iteration far enough that qT projection can trail).
    gen = attention_gen()

    def advance(n):
        for _ in range(n):
            if next(gen, None) is None:
                break

    bk = bqbk[:, HPG : 2 * HPG]
    xt_k0 = load_xt_quarter("xk", 0)
    xt_q_tiles[0] = load_xt_quarter("xq", 0)
    xt_v0 = load_xt_quarter("xv", 0)
    xt_k1 = load_xt_quarter("xk", 1)
    xt_v1 = load_xt_quarter("xv", 1)
    xt_k2 = load_xt_quarter("xk", 2)
    xt_v2 = load_xt_quarter("xv", 2)
    xt_k3 = load_xt_quarter("xk", 3)
    # (xv3 + xq1..3 are issued below once xt slots have been consumed)

    # Fine-grained prologue: projections are emitted in DMA-availability
    # order, spread across iteration 0's fires so the PE never holds a long
    # burst ahead of runnable attention work, and per-(head, sqq) so only
    # head 0's kT gates iteration 0 (h1/h2 trail for iterations 1-2; all xt
    # readers still finish within iteration 0's span, freeing slots for
    # xv3/xq1-3).
    # PV(g) of iteration 0 is emitted at fire g+3 (pop threshold 3), so vE
    # chunks 3g..3g+2 must be emitted before fire g+3.
    proj_qk_one(xt_k0, 0, 1, bk, kT, 0, 0)
    proj_q_group(xt_q_tiles[0], 0, 0, 0)
    advance(1)   # f1: g0 (chunks 0-2)
    proj_qk_one(xt_k0, 0, 1, bk, kT, 0, 1)
    proj_qk_one(xt_k0, 0, 1, bk, kT, 1, 0)
    advance(1)   # f2: g1
    proj_v(xt_v0, 0, (0, 1, 2))
    proj_qk_one(xt_k1, 1, 1, bk, kT, 0, 0)    # g2's chunk 8
    advance(1)   # f3: g2 + PV(g0)
    proj_v(xt_v0, 0, (3, 4, 5))
    proj_qk_one(xt_k0, 0, 1, bk, kT, 1, 1)
    advance(1)   # f4: g3 + PV(g1)
    proj_v(xt_v0, 0, (6, 7))
    proj_v(xt_v1, 1, (0,))
    proj_qk_one(xt_k1, 1, 1, bk, kT, 0, 1)    # g4's chunks 12-14
    proj_qk_one(xt_k0, 0, 1, bk, kT, 2, 0)
    advance(1)   # f5: g4 + PV(g2)
    proj_v(xt_v1, 1, (1, 2, 3))
    proj_qk_one(xt_k2, 2, 1, bk, kT, 0, 0)    # g5's chunks 16-17
    proj_qk_one(xt_k0, 0, 1, bk, kT, 2, 1)
    advance(1)   # f6: g5 + PV(g3)
    proj_v(xt_v1, 1, (4, 5, 6))
    proj_qk_one(xt_k2, 2, 1, bk, kT, 0, 1)    # g6's chunk 20
    proj_qk_one(xt_k1, 1, 1, bk, kT, 1, 0)
    advance(1)   # f7: g6 + PV(g4)
    xt_v3 = load_xt_quarter("xv", 3)
    proj_v(xt_v1, 1, (7,))
    proj_v(xt_v2, 2, (0, 1))
    proj_qk_one(xt_k1, 1, 1, bk, kT, 1, 1)
    advance(1)   # f8: g7 + PV(g5)
    proj_v(xt_v2, 2, (2, 3, 4))
    proj_qk_one(xt_k3, 3, 1, bk, kT, 0, 0)    # g8's chunk 24
    proj_qk_one(xt_k1, 1, 1, bk, kT, 2, 0)
    advance(1)   # f9: g8 + PV(g6)
    proj_v(xt_v2, 2, (5, 6, 7))
    proj_qk_one(xt_k3, 3, 1, bk, kT, 0, 1)    # g9's chunk 28
    proj_qk_one(xt_k1, 1, 1, bk, kT, 2, 1)
    xt_q_tiles[1] = load_xt_quarter("xq", 1)
    advance(1)   # f10: g9 + PV(g7)
    proj_v(xt_v3, 3, (0, 1, 2))
    proj_qk_one(xt_k2, 2, 1, bk, kT, 1, 0)
    proj_qk_one(xt_k2, 2, 1, bk, kT, 1, 1)
    xt_q_tiles[2] = load_xt_quarter("xq", 2)
    advance(1)   # f11: g10 + PV(g8) — iteration 0 groups complete
    proj_v(xt_v3, 3, (3, 4, 5, 6, 7))
    proj_q_group(xt_q_tiles[0], 0, 1, 0)      # iter1 = (h1, s0) at f12
    proj_qk_one(xt_k2, 2, 1, bk, kT, 2, 0)
    proj_qk_one(xt_k2, 2, 1, bk, kT, 2, 1)
    xt_q_tiles[3] = load_xt_quarter("xq", 3)
    advance(2)   # f12 (iter1 g0: carry PVs g9,g10 + fin), f13
    proj_qk_one(xt_k3, 3, 1, bk, kT, 1, 0)    # iter1 g8 reads at f20
    proj_qk_one(xt_k3, 3, 1, bk, kT, 1, 1)
    advance(2)   # f14, f15
    proj_qk_one(xt_k3, 3, 1, bk, kT, 2, 0)    # iter2 g8 reads at f31
    proj_qk_one(xt_k3, 3, 1, bk, kT, 2, 1)
    proj_q_group(xt_q_tiles[0], 0, 2, 0)      # iter2 = (h2, s0) at f23
    advance(7)   # f16-f22
    proj_q_group(xt_q_tiles[0], 0, 0, 1)      # iter3 = (h0, s1) at f34
    advance(2)   # f23, f24
    proj_q_group(xt_q_tiles[0], 0, 1, 1)      # iter4 at f45
    advance(5)   # f25-f29
    proj_q_group(xt_q_tiles[0], 0, 2, 1)      # iter5 at f56
    advance(5)   # f30-f34
    # qT quarters 1-3: one projection per 5 fires from ~f35 (deadline for
    # quarter q head h is fire 11*(6q+h), loose for every entry)
    for qq in (1, 2, 3):
        for h in range(HPG):
            for sqq in range(SQQ):
                proj_q_group(xt_q_tiles[qq], qq, h, sqq)
                advance(5)
    for _ in gen:
        pass


def _build():
    nc = bacc.Bacc("TRN2", target_bir_lowering=False, debug=False)
    mode = os.environ.get("BASS_FP8QK", "k")
    io = {}
    tensors = [
        ("xv", [NDC * S, 128], BF16), ("wv", [D, GD], BF16),
        ("bqbk_pk", [128, 2 * HPG], F32),
        ("bv_r", [1, GD], BF16), ("mask_pk", [128, NKC], F32),
    ]
    if mode in ("1", "k"):
        tensors.append(("wqk8", [128, 3 * 2 * 2 * HPG * 128], mybir.dt.float8e4))
        tensors.append(("xk", [3 * S, 128], BF16))
    else:
        tensors += [("xk", [NDC * S, 128], BF16), ("wk", [D, GD], BF16)]
    if mode == "1":
        tensors.append(("xq", [3 * S, 128], BF16))
    else:
        tensors += [("xq", [NDC * S, 128], BF16), ("wq", [D, GD], BF16)]
    for nm, shape, dt in tensors:
        io[nm] = nc.dram_tensor(nm, shape, dt, kind="ExternalInput").ap()
    io["out"] = nc.dram_tensor("out", [S, GD], F32, kind="ExternalOutput").ap()

    dup = int(os.environ.get("BASS_DUP", "1"))
    with tile.TileContext(nc) as tc:
        for _ in range(dup):
            with ExitStack() as ctx:
                _emit(ctx, tc, io)
    nc.compile()
    return nc


_NC = None


def _get_nc():
    global _NC
    if _NC is None:
        _NC = _build()
    return _NC


F8_NP = None


def _f8np():
    global F8_NP
    if F8_NP is None:
        from concourse import mybir as _mb
        F8_NP = _mb.dt.np(_mb.dt.float8e4)
    return F8_NP


def _pack_x8(a):
    # [S, D] f32 -> fp8 -> byte-pair uint16 carrier, d-pair-chunk-major
    # [3*S, 128] viewed as fp16 for the 2-byte xbar transpose
    a8 = np.ascontiguousarray(np.asarray(a, np.float32)).astype(_f8np())
    u = a8.view(np.uint8).reshape(S, 384, 2).view(np.uint16).reshape(S, 3, 128)
    return np.ascontiguousarray(
        u.transpose(1, 0, 2).reshape(3 * S, 128)
    ).view(np.float16)


def _pack_wqk8(Wq_c, Wk_c):
    # (ki, c, i, j, h, m) = 32 * W_i[256c + 2ki + j, h*64 + m%64]
    out = np.empty((128, 3, 2, 2, HPG, 128), np.float32)
    for i, W in ((0, Wq_c), (1, Wk_c)):
        Wr = (np.asarray(W, np.float32) * 32.0).reshape(3, 128, 2, HPG, DK)
        Wm = np.concatenate([Wr, Wr], axis=-1)        # [c, ki, j, h, 128]
        out[:, :, i] = Wm.transpose(1, 0, 2, 3, 4)    # [ki, c, j, h, 128]
    return np.ascontiguousarray(out.reshape(128, -1)).astype(_f8np())


def make_in_maps(query, key, value, mask, Wq, bq, Wk, bk, Wv, bv):
    mode = os.environ.get("BASS_FP8QK", "k")
    bf = lambda a: np.ascontiguousarray(a).astype(BF16_NP)
    bf3 = lambda a: np.ascontiguousarray(
        np.asarray(a).reshape(S, NDC, 128).transpose(1, 0, 2).reshape(NDC * S, 128)
    ).astype(BF16_NP)
    f32 = lambda a: np.ascontiguousarray(np.asarray(a, np.float32))
    in_maps = []
    for c in range(N_CORES):
        b, g = divmod(c, 4)
        cols = slice(g * GD, (g + 1) * GD)
        m = {
            "xv": bf3(value[b]),
            "wv": bf(Wv[:, cols]),
            "bqbk_pk": f32(np.tile(np.concatenate(
                [np.asarray(bq)[cols].reshape(HPG, DK).T,
                 np.asarray(bk)[cols].reshape(HPG, DK).T], axis=1), (2, 1))),
            "bv_r": bf(np.asarray(bv)[cols].reshape(1, GD)),
            "mask_pk": f32(np.asarray(mask)[b].reshape(NKC, 128).T),
        }
        if mode in ("1", "k"):
            m["xk"] = _pack_x8(key[b])
            m["wqk8"] = _pack_wqk8(
                np.asarray(Wq)[:, cols], np.asarray(Wk)[:, cols]
            )
        else:
            m["xk"] = bf3(key[b])
            m["wk"] = bf(Wk[:, cols])
        if mode == "1":
            m["xq"] = _pack_x8(query[b])
        else:
            m["xq"] = bf3(query[b])
            m["wq"] = bf(Wq[:, cols])
        in_maps.append(m)
    return in_maps


def kernel(query, key, value, mask, Wq, bq, Wk, bk, Wv, bv):
    query = np.asarray(query, np.float32)
    key = np.asarray(key, np.float32)
    value = np.asarray(value, np.float32)
    nc = _get_nc()
    in_maps = make_in_maps(query, key, value, mask, Wq, bq, Wk, bk, Wv, bv)
    res = run_bass_kernel_spmd(nc, in_maps, core_ids=list(range(N_CORES)))
    out = np.empty((B, S, D), np.float32)
    for c in range(N_CORES):
        b, g = divmod(c, 4)
        out[b, :, g * GD : (g + 1) * GD] = res.results[c]["out"]
    return out



# revision 55
# speedup vs baseline: 1.1537x; 1.0042x over previous
"""Multi-head attention (B=2, S=4096, D=768, H=12) on 8 Trainium2 cores.

Sharding: core c -> batch b = c // 4, head-triple g = c % 4 (heads 3g..3g+2).
Each core computes its QKV projections (columns of W for its heads) and
flash-style attention for its 3 heads, fully on-chip; no cross-core comms.
Host-side prep per core: slice batch/head-group, cast x/W to fp16 (f32
accumulation on device; fp16 over bf16 because all value ranges here are
tiny, ~8x lower quantization error at identical PE throughput). The K
projection inputs additionally go to fp8 (see below).

Per-core device kernel:
  - x^T tiles via xbar DMA-transpose straight from DRAM; ALL transposes are
    issued upfront in availability order (copies first on the same SP queue
    — mixed-queue issue interleaves copy/transpose at the shared HWDGE and
    every xbar-mode flip costs a drain), so the DMA engines stream
    continuously: kT quarters first, vE interleaved, xq last.
  - projections on PE produce qT/kT [64, 3, 4096] fp16 (duplicated on both
    partition halves) and v_ext [128, 32, 3, 65] (col 64 = ones so the PV
    matmul accumulates the softmax denominator as output row 64). The mask
    enters as a per-k scale em = exp(-1e4*(1-mask)) folded into v_ext
    (exact, including the denominator).
  - K projection (BASS_FP8QK="k", default) runs in fp8e4m3 DoubleRow: the
    host packs adjacent-d pairs of fp8 x into uint16 so the 2-byte xbar
    transpose yields the [128, 2, s] pair layout, and W*32 (lifted out of
    fp8's subnormal range; un-scaled in the bias-add) packed to match.
    Halves xk DMA bytes and K-proj PE streaming. Measured end-to-end err
    1.27e-2 l2 / 1.61e-2 absmax vs the 2e-2 gate. "1" extends it to Q
    (another -11us, but err 1.80e-2 l2 / 2.4e-2 absmax — too close).
  - attention iterations (h, sq) in h-minor order (sq-major), so qT quarter
    q is first needed at iteration 6q and the xq DMAs + qT projections trail
    far behind the kT/vE pipeline. Per iteration: 32 k-chunks in groups of
    3; QK^T -> PSUM, exp on ScalarE (scale=1/8) -> fp16 probs in SBUF, PV
    accumulate -> PSUM [65, 512] (positional start/stop flags); then
    PE-transpose and DVE normalize by the reciprocal of the denominator row.
  - prologue: projections are emitted fine-grained (per head/sqq; vE in
    chunk triplets) interleaved with iteration 0's exp groups, tracking DMA
    arrival; only (h0,s0)'s k+q projections gate the first exp; trailing qT
    projections spread one per 5 exp groups through iterations ~2-10.
  - PV emission lags exp by 3 groups (BASS_PVLAG=3 default): HW-measured
    -28us vs lag 2 (517 vs 546 same-process) — the PV tail decouples from
    the iteration transition; the 3-group carry lands at the next
    iteration's start where ACT is busy anyway.

Perf notes (HW-verified this series):
  - QK^T row tiling: odd k-chunks read the qT/kT replicas on partitions
    64-127 so consecutive K=64 matmuls run on different PE row groups
    (historically 589us -> 388us); reason for the column duplication.
  - split finalize: the pv->SBUF copy is emitted at the NEXT iteration's
    first exp group so the pv PSUM slot frees early; transposes+normalize
    one group later so they queue behind QK g1 on the PE.
  - projection PSUM tiles borrow scores-pool slots (NOT aux): aux holds the
    live pv accumulator, and a second rotating aux tenant serializes every
    projection against its DVE bias-add.
  - rejected on same-process HW A/B: DVE exp offload (BASS_DVE_NGROUPS=2,
    deg-2+double-squaring poly on the last 2 groups/iter) measured 579 vs
    553us — the DVE chain latency lands on the iteration transition.
    BASS_PAIR=1 (QK emission pairing across group boundaries) also negative.
  - PSUM budget: scores 2 bufs x 3 banks + pv 1 + tr 1 = 8 (full).
  - measurement: cross-process HW timing drifts ~+-8%; only same-process
    interleaved A/B slopes (ab.py / ab2.py) are trustworthy.
"""

import os
import sys

if "/opt/trn_rl_repo" not in sys.path:
    sys.path.insert(0, "/opt/trn_rl_repo")

from contextlib import ExitStack

import ml_dtypes
import numpy as np

import concourse.bass as bass
import concourse.tile as tile
from concourse import bacc, mybir
from concourse.bass_utils import run_bass_kernel_spmd
from concourse.masks import make_identity

F32 = mybir.dt.float32
# fp16 instead of bf16: all on-chip value ranges here are tiny (|x|<6,
# |W|<0.12, probs<8), so fp16's 10 mantissa bits cut quantization error ~4x
# at identical PE throughput (1 cycle/row) and xbar 2-byte transpose support
BF16 = mybir.dt.float16
AF = mybir.ActivationFunctionType
ALU = mybir.AluOpType
BF16_NP = np.float16

B, S, D, H, DK = 2, 4096, 768, 12, 64
N_CORES = 8
HPG = 3            # heads per core
GD = HPG * DK      # 192 output columns per core
SQ = 512           # q-chunk width
NSQ = S // SQ      # 8
KCW = 128          # k-chunk width
NKC = S // KCW     # 32
GRP = 3            # k-chunks per exp group (3 PSUM banks, double buffered)
NDC = D // 128     # 6 contraction chunks
QTR = S // 4       # transpose/projection pipeline granularity
SQQ = NSQ // 4     # q chunks per quarter
SCQ = NKC // 4     # s chunks per quarter


def _emit(ctx: ExitStack, tc: tile.TileContext, io: dict):
    nc = tc.nc

    const = ctx.enter_context(tc.tile_pool(name="const", bufs=1))
    # 8 slots: quarters xk0-3/xq0/xv0-2 are all live early; xv3 (9th alloc)
    # then reuses xk0's slot, whose readers finish by ~f6 — reuse of any
    # later slot (e.g. xq0's, read until ~f30) would deadlock the PE queue
    # against iteration 0's PV(g8).
    xt_pool = ctx.enter_context(tc.tile_pool(name="xt", bufs=8))
    proj = ctx.enter_context(tc.tile_pool(name="proj", bufs=1))
    scores_pool = ctx.enter_context(tc.tile_pool(name="scores", bufs=2, space="PSUM"))
    aux_psum = ctx.enter_context(tc.tile_pool(name="auxp", bufs=2, space="PSUM"))
    probs_pool = ctx.enter_context(tc.tile_pool(name="probs", bufs=5))
    outt_pool = ctx.enter_context(tc.tile_pool(name="outt", bufs=2))
    small = ctx.enter_context(tc.tile_pool(name="small", bufs=2))
    oslab_pool = ctx.enter_context(tc.tile_pool(name="oslab", bufs=3))
    # DVE-exp offload scratch: x tile, Horner ping-pong, and probs output
    dx_pool = ctx.enter_context(tc.tile_pool(name="dx", bufs=2))
    dh_pool = ctx.enter_context(tc.tile_pool(name="dh", bufs=3))
    prd_pool = ctx.enter_context(tc.tile_pool(name="prd", bufs=2))

    # ---- constants / small inputs (consolidated to limit 4KB slot padding) ----
    # mask -> per-k scale em = exp(-1e4 * (1 - mask)), [128, 32] (p, kchunk).
    # Emitted FIRST so the ACT exp-table load lands at the head of the queues.
    # All const DMAs go on the SAME queue (SP) that later issues the x
    # DMA-transposes: the shared HWDGE serializes globally and every
    # copy<->transpose transition costs an xbar-mode drain, so mixed-queue
    # issue (copies from ACT, transposes from SP) interleaved them worst-case.
    mask_em = const.tile([128, 65], F32, name="mask_em")
    mask_t = mask_em[:, 0:32]
    em_sb = mask_em[:, 32:64]
    neg1e4 = mask_em[:, 64:65]
    nc.gpsimd.memset(neg1e4, -10000.0)
    nc.sync.dma_start(mask_t, io["mask_pk"][:])
    nc.scalar.activation(em_sb, mask_t, AF.Exp, scale=10000.0, bias=neg1e4)

    # "0": all-fp16 projections. "1": q AND k projections via fp8 DoubleRow
    # (hw-measured end-to-end rel err 1.8e-2 — too close to the 2e-2 gate).
    # "k": only the K projection in fp8 (err ~1.2e-2 l2 / 1.5e-2 absmax,
    # comfortable margin) at half the PE savings.
    FP8MODE = __import__("os").environ.get("BASS_FP8QK", "k")
    FP8QK = FP8MODE in ("1", "k")
    F8 = mybir.dt.float8e4

    def fp8_for(wi):
        return FP8MODE == "1" or (FP8MODE == "k" and wi == 1)

    if FP8QK:
        # q/k weights as fp8 DoubleRow pairs, host-packed in tile layout:
        # (ki, c, i, j, h, m) = 32*W_i[256c + 2ki + j, h*64 + m%64]
        # (x32 lifts W sigma=0.02 out of fp8's subnormal range; the bias-add
        # multiplies the PSUM result by 1/32)
        w8 = const.tile([128, 3, 2, 2, HPG, 128], F8, name="w8")
        nc.sync.dma_start(
            w8[:],
            io["wqk8"].rearrange(
                "p (c i j h m) -> p c i j h m", c=3, i=2, j=2, h=HPG
            ),
        )
    # fp16 weight slabs: v always; q and/or k when their projection is fp16
    fp16_w = [(2, "wv")]
    if not fp8_for(0):
        fp16_w.append((0, "wq"))
    if not fp8_for(1):
        fp16_w.append((1, "wk"))
    w_all = const.tile([128, NDC, 3 * GD], BF16, name="w_all")
    for i, nm in fp16_w:
        nc.sync.dma_start(
            w_all[:, :, i * GD : (i + 1) * GD],
            io[nm].rearrange("(dc p) n -> p dc n", p=128),
        )
    wv_sb = w_all[:, :, 2 * GD : 3 * GD]

    if not (fp8_for(0) and fp8_for(1)):
        # q/k weights with each head's 64 columns duplicated (projection then
        # replicates qT/kT on both partition halves at no extra PE cost)
        w_dup = const.tile([128, NDC, 2, HPG, 128], BF16, name="w_dup")
        for i, _nm in fp16_w:
            if i == 2:
                continue
            for h in range(HPG):
                for rep in range(2):
                    nc.vector.tensor_copy(
                        w_dup[:, :, i, h, rep * DK : (rep + 1) * DK],
                        w_all[:, :, i * GD + h * DK : i * GD + (h + 1) * DK],
                    )

    bqbk = const.tile([128, 2 * HPG], F32, name="bqbk")
    nc.sync.dma_start(bqbk[:], io["bqbk_pk"][:])

    bfpack = const.tile([1, 320], BF16, name="bfpack")
    nc.gpsimd.memset(bfpack[:, 0:128], 1.0)
    nc.sync.dma_start(bfpack[:, 128 : 128 + GD], io["bv_r"][:])
    ones_row = bfpack[:, 0:128]
    bv_sb = bfpack[:, 128 : 128 + GD]

    ident = const.tile([128, 128], F32, name="ident")
    make_identity(nc, ident[:])

    # ---- persistent projection outputs (qT/kT replicated on both halves) ----
    qT = proj.tile([128, HPG, S], BF16, name="qT")
    kT = proj.tile([128, HPG, S], BF16, name="kT")
    vE = proj.tile([128, NKC, HPG, DK + 1], BF16, name="vE")
    nc.gpsimd.memset(vE[:], 1.0)  # ones col 64; data cols overwritten below

    # ---- per-quarter: transpose + project ----
    def load_xt_quarter(nm, qq):
        # host supplies x d-chunk-major [6*4096, 128] so each xbar transpose
        # reads a fully contiguous [1024, 128] block. In FP8QK mode, xq/xk
        # arrive byte-packed (two fp8 d-neighbors per uint16 element): 3
        # chunks of 128 pair-columns, half the DMA bytes.
        packed = (nm == "xk" and FP8QK) or (nm == "xq" and FP8MODE == "1")
        nch = 3 if packed else NDC
        xt = xt_pool.tile([128, nch, QTR], BF16, tag="xt", name=f"xt_{nm}_{qq}")
        for dc in range(nch):
            base = dc * S + qq * QTR
            nc.sync.dma_start(
                out=xt[:, dc, :], in_=io[nm][base : base + QTR, :],
                transpose=True,
            )
        return xt

    def proj_qk_one(xt, qq, wi, bias, dst, h, sqq):
        # scores-pool slot (not aux): aux holds the live pv accumulator, so a
        # second rotating tenant there would serialize every projection
        # against its DVE bias-add read
        sq = qq * SQQ + sqq
        ps = scores_pool.tile([128, SQ], F32, tag="scores", name=f"ps_{wi}_{qq}_{h}_{sqq}")
        if fp8_for(wi):
            # fp8 DoubleRow: 3 contraction chunks of 256 d (pairs d=256c+2ki+j
            # matching the byte-packed transpose and the host w8 layout);
            # each chunk streams N=512 at 0.5 cycles/row
            x8 = xt[:].bitcast(F8).rearrange("p c (s j) -> p c j s", j=2)
            for c in range(3):
                nc.tensor.matmul(
                    ps[:],
                    lhsT=w8[:, c, wi, :, h, :],
                    rhs=x8[:, c, :, sqq * SQ : (sqq + 1) * SQ],
                    start=(c == 0),
                    stop=(c == 2),
                    perf_mode=mybir.MatmulPerfMode.DoubleRow,
                )
            # undo the x32 weight scale, then add bias
            nc.vector.tensor_scalar(
                dst[:, h, sq * SQ : (sq + 1) * SQ], ps[:],
                0.03125, bias[:, h : h + 1], ALU.mult, ALU.add,
            )
        else:
            for dc in range(NDC):
                nc.tensor.matmul(
                    ps[:],
                    lhsT=w_dup[:, dc, wi, h, :],
                    rhs=xt[:, dc, sqq * SQ : (sqq + 1) * SQ],
                    start=(dc == 0),
                    stop=(dc == NDC - 1),
                )
            nc.vector.tensor_scalar(
                dst[:, h, sq * SQ : (sq + 1) * SQ], ps[:],
                bias[:, h : h + 1], None, ALU.add,
            )

    def proj_qk(xt, qq, wi, bias, dst, skip=None):
        for h in range(HPG):
            for sqq in range(SQQ):
                if skip is not None and (h, sqq) in skip:
                    continue
                proj_qk_one(xt, qq, wi, bias, dst, h, sqq)

    def proj_v(xt, qq, scqs=None):
        for scq in (range(SCQ) if scqs is None else scqs):
            sc = qq * SCQ + scq
            ps = scores_pool.tile([128, GD], F32, tag="scores", name=f"psv_{qq}_{scq}")
            for dc in range(NDC):
                nc.tensor.matmul(
                    ps[:],
                    lhsT=xt[:, dc, scq * 128 : (scq + 1) * 128],
                    rhs=wv_sb[:, dc, :],
                    start=(dc == 0),
                    stop=False,
                )
            nc.tensor.matmul(
                ps[:], lhsT=ones_row[:, 0:128], rhs=bv_sb[:], start=False, stop=True
            )
            for h in range(HPG):
                nc.vector.tensor_copy(
                    vE[:, sc, h, 0:DK], ps[:, h * DK : (h + 1) * DK]
                )
            # fold mask scale into v and the denominator ones column
            nc.vector.tensor_scalar(
                vE[:, sc, :, :], vE[:, sc, :, :], em_sb[:, sc : sc + 1], None,
                ALU.mult,
            )

    def proj_q_group(xt, qq, h, sqq):
        proj_qk_one(xt, qq, 0, bqbk[:, 0:HPG], qT, h, sqq)

    # ---- attention ----
    groups = []
    g0 = 0
    while g0 < NKC:
        groups.append((g0, min(GRP, NKC - g0)))
        g0 += GRP

    # ScalarE exp is the kernel's critical engine in steady state. Offload
    # the LAST `DVE_NGROUPS` exp groups of each iteration (latest PV
    # deadlines) to a DVE polynomial: exp(s/8) = h(y)^4 with y = s/32 and
    # h a degree-2 least-squares fit of exp on y in [-0.5, 0.5]. Squaring
    # twice keeps probs nonnegative by construction. 6 DVE ops per group
    # (1 PSUM-read + 5 fp16-SBUF ops) vs 1 ACT op; worth it because DVE is
    # ~80% idle while ACT is the wall. Adds ~3.7e-3 relative error on the
    # offloaded 5/32 of keys (budget 2e-2).
    # A/B on HW (same-process, interleaved): offload=2 groups measured ~26us
    # SLOWER (579 vs 553) despite ACT being the busiest engine — the DVE
    # chain latency sits on the iteration-transition critical path. Off by
    # default.
    DVE_NGROUPS = int(__import__("os").environ.get("BASS_DVE_NGROUPS", "0"))
    C0, C1, C2 = 1.00148143, 1.02379966, 0.48757841

    def emit_dve_exp(sc, pr, w, it, gi):
        yt = dx_pool.tile([128, w], BF16, tag="dx", name=f"y_{it}_{gi}")
        nc.vector.tensor_scalar(yt[:], sc[:], 0.03125, None, ALU.mult)
        t1 = dh_pool.tile([128, w], BF16, tag="dh", name=f"t1_{it}_{gi}")
        nc.vector.tensor_scalar(t1[:], yt[:], C2, C1, ALU.mult, ALU.add)
        t2 = dh_pool.tile([128, w], BF16, tag="dh", name=f"t2_{it}_{gi}")
        nc.vector.tensor_tensor(t2[:], t1[:], yt[:], ALU.mult)
        t3 = dh_pool.tile([128, w], BF16, tag="dh", name=f"t3_{it}_{gi}")
        nc.vector.tensor_scalar(t3[:], t2[:], C0, None, ALU.add)
        t4 = dh_pool.tile([128, w], BF16, tag="dh", name=f"t4_{it}_{gi}")
        nc.vector.tensor_tensor(t4[:], t3[:], t3[:], ALU.mult)
        nc.vector.tensor_tensor(pr[:], t4[:], t4[:], ALU.mult)

    pending = None  # finalize closure for the previous (h, sq)

    # finalize split in two: part 1 (the pv->SBUF copy, which frees the pv
    # PSUM slot) fires at gi==0 so it lands in the DVE queue BEFORE the
    # ~10us exp-poly chain; part 2 (PE transposes + normalize + store) at
    # gi==1 so the transposes sit behind QK g1 in the PE queue and never
    # stall on the copy.
    def make_fin_copy(pv, h, sq):
        ot = outt_pool.tile([DK + 1, SQ], F32, tag="outt", name=f"ot_{h}_{sq}")

        def fin1():
            nc.vector.tensor_copy(ot[:], pv[:])
        return ot, fin1

    def make_finalize(ot, h, sq):
        def fin():
            tr = aux_psum.tile([128, 4 * (DK + 1)], F32, tag="aux", name=f"tr_{h}_{sq}")
            for t in range(4):
                nc.tensor.transpose(
                    tr[:, t * (DK + 1) : (t + 1) * (DK + 1)],
                    ot[:, t * 128 : (t + 1) * 128],
                    ident[0 : DK + 1, 0 : DK + 1],
                )
            rc = small.tile([128, 4], F32, tag="recip", name=f"rc_{h}_{sq}")
            osl = oslab_pool.tile([128, 4, DK], F32, tag="oslab", name=f"os_{h}_{sq}")
            for t in range(4):
                nc.vector.reciprocal(
                    rc[:, t : t + 1], tr[:, t * (DK + 1) + DK : t * (DK + 1) + DK + 1]
                )
                nc.vector.tensor_scalar(
                    osl[:, t, :],
                    tr[:, t * (DK + 1) : t * (DK + 1) + DK],
                    rc[:, t : t + 1],
                    None,
                    ALU.mult,
                )
            nc.gpsimd.dma_start(
                out=io["out"].rearrange(
                    "(sq t p) n -> sq p t n", sq=NSQ, t=4, p=128
                )[sq, :, :, h * DK : (h + 1) * DK],
                in_=osl[:],
            )
        return fin

    # Boundary tasks: kept as an (empty by default) hook used by the carry
    # emission point inside attention_gen.
    boundary_tasks = {}
    xt_q_tiles = {}

    def attention_gen():
        nonlocal_pending = [None]

        def emit_pv(pv, h, grp, pvst):
            # start/stop are positional (pvst counts PV matmuls emitted for
            # this accumulator): chunk emission order is permuted when DVE
            # exp groups are deferred, so kc == 0 is not necessarily first.
            p0, plen, ppr = grp
            for j in range(plen):
                kc = p0 + j
                nc.tensor.matmul(
                    pv[:],
                    lhsT=vE[:, kc, h, :],
                    rhs=ppr[:, j * SQ : (j + 1) * SQ],
                    start=(pvst[0] == 0),
                    stop=(pvst[0] == NKC - 1),
                )
                pvst[0] += 1

        carry = None  # (pv, h, [groups]) tail-PV work carried across iterations
        it = 0
        # PV pop threshold (lag in exp groups before a group's PV is emitted).
        # 3 measured -28us on HW vs 2 (sim agrees: the PV tail decouples from
        # the iteration transition; carry grows to 3 groups, emitted at the
        # next iteration's start where ACT is busy anyway).
        PVLAG = int(__import__("os").environ.get("BASS_PVLAG", "3"))
        ITER_HMINOR = __import__("os").environ.get("BASS_HMINOR", "1") == "1"
        # h-minor / sq-major order: qT quarter q is first needed at iteration
        # 3*2q, so the xq DMAs and qT projections can trail far behind the
        # kT/vE pipeline instead of gating it.
        if ITER_HMINOR:
            iters = [(h, sq) for sq in range(NSQ) for h in range(HPG)]
        else:
            iters = [(h, sq) for h in range(HPG) for sq in range(NSQ)]
        for h, sq in iters:
            # From iteration 2 on, the DVE-offloaded groups (the LAST k-chunk
            # groups) are FIRED FIRST: their scores slots free early (from the
            # previous iteration), and their ~5us DVE poly chains overlap this
            # iteration's ACT burst instead of delaying the carry PVs.
            offload = DVE_NGROUPS if it >= 2 else 0
            if offload:
                act_groups = groups[-offload:] + groups[:-offload]
            else:
                act_groups = groups
            dve_set = set(range(offload))
            pv = aux_psum.tile([DK + 1, SQ], F32, tag="aux", name=f"pv_{h}_{sq}")
            pvst = [0]
            ready = []  # (kc0, glen, probs) groups awaiting PV emission
            defer = []  # (eligible_fire_idx, grp) DVE groups awaiting poly
            # QK matmuls are emitted in strict (even, odd) kc pairs ACROSS
            # group boundaries so every matmul lands adjacent to its
            # opposite-row-group partner in the PE queue and the two K=64
            # halves run concurrently (PV blocks between groups would
            # otherwise orphan each group's 3rd chunk)
            chunk_list = []
            for gi, (kc0, glen) in enumerate(act_groups):
                for j in range(glen):
                    chunk_list.append((kc0 + j, gi, j))
            sc_tiles = {}
            filled = [0] * len(act_groups)
            fired = 0
            ci = 0
            # no pairing in iteration 0: its one-chunk lookahead would hold a
            # live scores tile across the prologue's advance() points, where
            # proj_q_group borrows slots from the same pool
            # A/B on HW: pairing measured neutral-to-worse (478us vs 450us
            # best-valid samples) — likely the 1-chunk lookahead couples the
            # PE to the previous exp via the scores double-buffer. Opt-in.
            do_pair = __import__("os").environ.get("BASS_PAIR", "0") == "1"
            pair_n = 2 if (it > 0 and do_pair) else 1
            while ci < len(chunk_list):
                for _ in range(pair_n):
                    if ci >= len(chunk_list):
                        break
                    kc, gi, j = chunk_list[ci]
                    ci += 1
                    if gi not in sc_tiles:
                        sc_tiles[gi] = scores_pool.tile(
                            [128, act_groups[gi][1] * SQ], F32, tag="scores",
                            name=f"sc_{h}_{sq}_{gi}",
                        )
                    ho = 64 * (kc % 2)
                    nc.tensor.matmul(
                        sc_tiles[gi][:, j * SQ : (j + 1) * SQ],
                        lhsT=kT[ho : ho + DK, h, kc * KCW : (kc + 1) * KCW],
                        rhs=qT[ho : ho + DK, h, sq * SQ : (sq + 1) * SQ],
                        start=True,
                        stop=True,
                    )
                    filled[gi] += 1
                while fired < len(act_groups) and (
                    filled[fired] == act_groups[fired][1]
                ):
                    gi = fired
                    kc0, glen = act_groups[gi]
                    pr = probs_pool.tile(
                        [128, glen * SQ], BF16, tag="probs",
                        name=f"pr_{h}_{sq}_{gi}",
                    )
                    if gi in dve_set:
                        emit_dve_exp(
                            sc_tiles.pop(gi), pr, glen * SQ, it, gi
                        )
                        defer.append((gi + 4, (kc0, glen, pr)))
                    else:
                        nc.scalar.activation(
                            pr[:], sc_tiles.pop(gi)[:], AF.Exp, scale=0.125
                        )
                        ready.append((kc0, glen, pr))
                    while defer and defer[0][0] <= gi:
                        ready.append(defer.pop(0)[1])
                    if gi == 0:
                        if carry is not None:
                            cpv, ch, cgrps, cpvst = carry
                            for grp in cgrps:
                                emit_pv(cpv, ch, grp, cpvst)
                            carry = None
                            for fn in boundary_tasks.get(it - 1, ()):
                                fn()
                        # free the previous pv PSUM slot (DVE copy) BEFORE
                        # the iteration's poly chains enter the DVE queue
                        if nonlocal_pending[0] is not None:
                            nonlocal_pending[0][0]()
                    if gi == 1 and nonlocal_pending[0] is not None:
                        nonlocal_pending[0][1]()
                        nonlocal_pending[0] = None
                    # iteration 0 holds one extra group in flight: its vE
                    # quarters are still streaming in, so the deeper lag keeps
                    # PV emission behind the xv DMA wavefront
                    if len(ready) >= (max(3, PVLAG) if it == 0 else PVLAG):
                        emit_pv(pv, h, ready.pop(0), pvst)
                    fired += 1
                    yield (h, sq, gi)
            carry_grps = list(ready) + [g for _, g in defer]
            carry = (pv, h, carry_grps, pvst)
            ot, fin1 = make_fin_copy(pv, h, sq)
            nonlocal_pending[0] = (fin1, make_finalize(ot, h, sq))
            it += 1

        cpv, ch, cgrps, cpvst = carry
        for grp in cgrps:
            emit_pv(cpv, ch, grp, cpvst)
        nonlocal_pending[0][0]()
        nonlocal_pending[0][1]()

    # All x DMA-transposes are issued upfront in availability order so the
    # DMA engines stream continuously from t=0 (the quarter-chasing scheme
    # left them ~50% idle over a 100us window). kT quarters land first (they
    # gate iteration 0's exp groups), vE quarters interleave (they gate the
    # trailing PV), and xq quarters come last (h-minor iteration order defers
    # the first sq>=2 iteration far enough that qT projection can trail).
    gen = attention_gen()

    def advance(n):
        for _ in range(n):
            if next(gen, None) is None:
                break

    bk = bqbk[:, HPG : 2 * HPG]
    xt_k0 = load_xt_quarter("xk", 0)
    xt_q_tiles[0] = load_xt_quarter("xq", 0)
    xt_v0 = load_xt_quarter("xv", 0)
    xt_k1 = load_xt_quarter("xk", 1)
    xt_v1 = load_xt_quarter("xv", 1)
    xt_k2 = load_xt_quarter("xk", 2)
    xt_v2 = load_xt_quarter("xv", 2)
    xt_k3 = load_xt_quarter("xk", 3)
    # (xv3 + xq1..3 are issued below once xt slots have been consumed)

    # Fine-grained prologue: projections are emitted in DMA-availability
    # order, spread across iteration 0's fires so the PE never holds a long
    # burst ahead of runnable attention work, and per-(head, sqq) so only
    # head 0's kT gates iteration 0 (h1/h2 trail for iterations 1-2; all xt
    # readers still finish within iteration 0's span, freeing slots for
    # xv3/xq1-3).
    # PV(g) of iteration 0 is emitted at fire g+3 (pop threshold 3), so vE
    # chunks 3g..3g+2 must be emitted before fire g+3.
    proj_qk_one(xt_k0, 0, 1, bk, kT, 0, 0)
    proj_q_group(xt_q_tiles[0], 0, 0, 0)
    advance(1)   # f1: g0 (chunks 0-2)
    proj_qk_one(xt_k0, 0, 1, bk, kT, 0, 1)
    proj_qk_one(xt_k0, 0, 1, bk, kT, 1, 0)
    advance(1)   # f2: g1
    proj_v(xt_v0, 0, (0, 1, 2))
    proj_qk_one(xt_k1, 1, 1, bk, kT, 0, 0)    # g2's chunk 8
    advance(1)   # f3: g2 + PV(g0)
    proj_v(xt_v0, 0, (3, 4, 5))
    proj_qk_one(xt_k0, 0, 1, bk, kT, 1, 1)
    advance(1)   # f4: g3 + PV(g1)
    proj_v(xt_v0, 0, (6, 7))
    proj_v(xt_v1, 1, (0,))
    proj_qk_one(xt_k1, 1, 1, bk, kT, 0, 1)    # g4's chunks 12-14
    proj_qk_one(xt_k0, 0, 1, bk, kT, 2, 0)
    advance(1)   # f5: g4 + PV(g2)
    proj_v(xt_v1, 1, (1, 2, 3))
    proj_qk_one(xt_k2, 2, 1, bk, kT, 0, 0)    # g5's chunks 16-17
    proj_qk_one(xt_k0, 0, 1, bk, kT, 2, 1)
    advance(1)   # f6: g5 + PV(g3)
    proj_v(xt_v1, 1, (4, 5, 6))
    proj_qk_one(xt_k2, 2, 1, bk, kT, 0, 1)    # g6's chunk 20
    proj_qk_one(xt_k1, 1, 1, bk, kT, 1, 0)
    advance(1)   # f7: g6 + PV(g4)
    xt_v3 = load_xt_quarter("xv", 3)
    proj_v(xt_v1, 1, (7,))
    proj_v(xt_v2, 2, (0, 1))
    proj_qk_one(xt_k1, 1, 1, bk, kT, 1, 1)
    advance(1)   # f8: g7 + PV(g5)
    proj_v(xt_v2, 2, (2, 3, 4))
    proj_qk_one(xt_k3, 3, 1, bk, kT, 0, 0)    # g8's chunk 24
    proj_qk_one(xt_k1, 1, 1, bk, kT, 2, 0)
    advance(1)   # f9: g8 + PV(g6)
    proj_v(xt_v2, 2, (5, 6, 7))
    proj_qk_one(xt_k3, 3, 1, bk, kT, 0, 1)    # g9's chunk 28
    proj_qk_one(xt_k1, 1, 1, bk, kT, 2, 1)
    xt_q_tiles[1] = load_xt_quarter("xq", 1)
    advance(1)   # f10: g9 + PV(g7)
    proj_v(xt_v3, 3, (0, 1, 2))
    proj_qk_one(xt_k2, 2, 1, bk, kT, 1, 0)
    proj_qk_one(xt_k2, 2, 1, bk, kT, 1, 1)
    xt_q_tiles[2] = load_xt_quarter("xq", 2)
    advance(1)   # f11: g10 + PV(g8) — iteration 0 groups complete
    proj_v(xt_v3, 3, (3, 4, 5, 6, 7))
    proj_q_group(xt_q_tiles[0], 0, 1, 0)      # iter1 = (h1, s0) at f12
    proj_qk_one(xt_k2, 2, 1, bk, kT, 2, 0)
    proj_qk_one(xt_k2, 2, 1, bk, kT, 2, 1)
    xt_q_tiles[3] = load_xt_quarter("xq", 3)
    advance(2)   # f12 (iter1 g0: carry PVs g9,g10 + fin), f13
    proj_qk_one(xt_k3, 3, 1, bk, kT, 1, 0)    # iter1 g8 reads at f20
    proj_qk_one(xt_k3, 3, 1, bk, kT, 1, 1)
    advance(2)   # f14, f15
    proj_qk_one(xt_k3, 3, 1, bk, kT, 2, 0)    # iter2 g8 reads at f31
    proj_qk_one(xt_k3, 3, 1, bk, kT, 2, 1)
    proj_q_group(xt_q_tiles[0], 0, 2, 0)      # iter2 = (h2, s0) at f23
    advance(7)   # f16-f22
    proj_q_group(xt_q_tiles[0], 0, 0, 1)      # iter3 = (h0, s1) at f34
    advance(2)   # f23, f24
    proj_q_group(xt_q_tiles[0], 0, 1, 1)      # iter4 at f45
    advance(5)   # f25-f29
    proj_q_group(xt_q_tiles[0], 0, 2, 1)      # iter5 at f56
    advance(5)   # f30-f34
    # qT quarters 1-3: one projection per 5 fires from ~f35 (deadline for
    # quarter q head h is fire 11*(6q+h), loose for every entry)
    for qq in (1, 2, 3):
        for h in range(HPG):
            for sqq in range(SQQ):
                proj_q_group(xt_q_tiles[qq], qq, h, sqq)
                advance(5)
    for _ in gen:
        pass


def _build():
    nc = bacc.Bacc("TRN2", target_bir_lowering=False, debug=False)
    mode = os.environ.get("BASS_FP8QK", "k")
    io = {}
    tensors = [
        ("xv", [NDC * S, 128], BF16), ("wv", [D, GD], BF16),
        ("bqbk_pk", [128, 2 * HPG], F32),
        ("bv_r", [1, GD], BF16), ("mask_pk", [128, NKC], F32),
    ]
    if mode in ("1", "k"):
        tensors.append(("wqk8", [128, 3 * 2 * 2 * HPG * 128], mybir.dt.float8e4))
        tensors.append(("xk", [3 * S, 128], BF16))
    else:
        tensors += [("xk", [NDC * S, 128], BF16), ("wk", [D, GD], BF16)]
    if mode == "1":
        tensors.append(("xq", [3 * S, 128], BF16))
    else:
        tensors += [("xq", [NDC * S, 128], BF16), ("wq", [D, GD], BF16)]
    for nm, shape, dt in tensors:
        io[nm] = nc.dram_tensor(nm, shape, dt, kind="ExternalInput").ap()
    io["out"] = nc.dram_tensor("out", [S, GD], F32, kind="ExternalOutput").ap()

    dup = int(os.environ.get("BASS_DUP", "1"))
    with tile.TileContext(nc) as tc:
        for _ in range(dup):
            with ExitStack() as ctx:
                _emit(ctx, tc, io)
    nc.compile()
    return nc


_NC = None


def _get_nc():
    global _NC
    if _NC is None:
        _NC = _build()
    return _NC


F8_NP = None


def _f8np():
    global F8_NP
    if F8_NP is None:
        from concourse import mybir as _mb
        F8_NP = _mb.dt.np(_mb.dt.float8e4)
    return F8_NP


def _pack_x8(a):
    # [S, D] f32 -> fp8 -> byte-pair uint16 carrier, d-pair-chunk-major
    # [3*S, 128] viewed as fp16 for the 2-byte xbar transpose
    a8 = np.ascontiguousarray(np.asarray(a, np.float32)).astype(_f8np())
    u = a8.view(np.uint8).reshape(S, 384, 2).view(np.uint16).reshape(S, 3, 128)
    return np.ascontiguousarray(
        u.transpose(1, 0, 2).reshape(3 * S, 128)
    ).view(np.float16)


def _pack_wqk8(Wq_c, Wk_c):
    # (ki, c, i, j, h, m) = 32 * W_i[256c + 2ki + j, h*64 + m%64]
    out = np.empty((128, 3, 2, 2, HPG, 128), np.float32)
    for i, W in ((0, Wq_c), (1, Wk_c)):
        Wr = (np.asarray(W, np.float32) * 32.0).reshape(3, 128, 2, HPG, DK)
        Wm = np.concatenate([Wr, Wr], axis=-1)        # [c, ki, j, h, 128]
        out[:, :, i] = Wm.transpose(1, 0, 2, 3, 4)    # [ki, c, j, h, 128]
    return np.ascontiguousarray(out.reshape(128, -1)).astype(_f8np())


def make_in_maps(query, key, value, mask, Wq, bq, Wk, bk, Wv, bv):
    mode = os.environ.get("BASS_FP8QK", "k")
    bf = lambda a: np.ascontiguousarray(a).astype(BF16_NP)
    bf3 = lambda a: np.ascontiguousarray(
        np.asarray(a).reshape(S, NDC, 128).transpose(1, 0, 2).reshape(NDC * S, 128)
    ).astype(BF16_NP)
    f32 = lambda a: np.ascontiguousarray(np.asarray(a, np.float32))
    in_maps = []
    for c in range(N_CORES):
        b, g = divmod(c, 4)
        cols = slice(g * GD, (g + 1) * GD)
        m = {
            "xv": bf3(value[b]),
            "wv": bf(Wv[:, cols]),
            "bqbk_pk": f32(np.tile(np.concatenate(
                [np.asarray(bq)[cols].reshape(HPG, DK).T,
                 np.asarray(bk)[cols].reshape(HPG, DK).T], axis=1), (2, 1))),
            "bv_r": bf(np.asarray(bv)[cols].reshape(1, GD)),
            "mask_pk": f32(np.asarray(mask)[b].reshape(NKC, 128).T),
        }
        if mode in ("1", "k"):
            m["xk"] = _pack_x8(key[b])
            m["wqk8"] = _pack_wqk8(
                np.asarray(Wq)[:, cols], np.asarray(Wk)[:, cols]
            )
        else:
            m["xk"] = bf3(key[b])
            m["wk"] = bf(Wk[:, cols])
        if mode == "1":
            m["xq"] = _pack_x8(query[b])
        else:
            m["xq"] = bf3(query[b])
            m["wq"] = bf(Wq[:, cols])
        in_maps.append(m)
    return in_maps


def kernel(query, key, value, mask, Wq, bq, Wk, bk, Wv, bv):
    query = np.asarray(query, np.float32)
    key = np.asarray(key, np.float32)
    value = np.asarray(value, np.float32)
    nc = _get_nc()
    in_maps = make_in_maps(query, key, value, mask, Wq, bq, Wk, bk, Wv, bv)
    res = run_bass_kernel_spmd(nc, in_maps, core_ids=list(range(N_CORES)))
    out = np.empty((B, S, D), np.float32)
    for c in range(N_CORES):
        b, g = divmod(c, 4)
        out[b, :, g * GD : (g + 1) * GD] = res.results[c]["out"]
    return out



# revision 56
# speedup vs baseline: 1.1560x; 1.0020x over previous
"""Multi-head attention (B=2, S=4096, D=768, H=12) on 8 Trainium2 cores.

Sharding: core c -> batch b = c // 4, head-triple g = c % 4 (heads 3g..3g+2).
Each core computes its QKV projections (columns of W for its heads) and
flash-style attention for its 3 heads, fully on-chip; no cross-core comms.
Host-side prep per core: slice batch/head-group, cast x/W to fp16 (f32
accumulation on device; fp16 over bf16 because all value ranges here are
tiny, ~8x lower quantization error at identical PE throughput). The K
projection inputs additionally go to fp8 (see below).

Per-core device kernel:
  - x^T tiles via xbar DMA-transpose straight from DRAM; ALL transposes are
    issued upfront in availability order (copies first on the same SP queue
    — mixed-queue issue interleaves copy/transpose at the shared HWDGE and
    every xbar-mode flip costs a drain), so the DMA engines stream
    continuously: kT quarters first, vE interleaved, xq last.
  - projections on PE produce qT/kT [64, 3, 4096] fp16 (duplicated on both
    partition halves) and v_ext [128, 32, 3, 65] (col 64 = ones so the PV
    matmul accumulates the softmax denominator as output row 64). The mask
    enters as a per-k scale em = exp(-1e4*(1-mask)) folded into v_ext
    (exact, including the denominator).
  - K projection (BASS_FP8QK="k", default) runs in fp8e4m3 DoubleRow: the
    host packs adjacent-d pairs of fp8 x into uint16 so the 2-byte xbar
    transpose yields the [128, 2, s] pair layout, and W*32 (lifted out of
    fp8's subnormal range; un-scaled in the bias-add) packed to match.
    Halves xk DMA bytes and K-proj PE streaming. Measured end-to-end err
    1.27e-2 l2 / 1.61e-2 absmax vs the 2e-2 gate. "1" extends it to Q
    (another -11us, but err 1.80e-2 l2 / 2.4e-2 absmax — too close).
  - attention iterations (h, sq) in h-minor order (sq-major), so qT quarter
    q is first needed at iteration 6q and the xq DMAs + qT projections trail
    far behind the kT/vE pipeline. Per iteration: 32 k-chunks in groups of
    3; QK^T -> PSUM, exp on ScalarE (scale=1/8) -> fp16 probs in SBUF, PV
    accumulate -> PSUM [65, 512] (positional start/stop flags); then
    PE-transpose and DVE normalize by the reciprocal of the denominator row.
  - prologue: projections are emitted fine-grained (per head/sqq; vE in
    chunk triplets) interleaved with iteration 0's exp groups, tracking DMA
    arrival; only (h0,s0)'s k+q projections gate the first exp; trailing qT
    projections spread one per 5 exp groups through iterations ~2-10.
  - PV emission lags exp by 3 groups (BASS_PVLAG=3 default): HW-measured
    -28us vs lag 2 (517 vs 546 same-process) — the PV tail decouples from
    the iteration transition; the 3-group carry lands at the next
    iteration's start where ACT is busy anyway.

Perf notes (HW-verified this series):
  - QK^T row tiling: odd k-chunks read the qT/kT replicas on partitions
    64-127 so consecutive K=64 matmuls run on different PE row groups
    (historically 589us -> 388us); reason for the column duplication.
  - split finalize: the pv->SBUF copy is emitted at the NEXT iteration's
    first exp group so the pv PSUM slot frees early; transposes+normalize
    one group later so they queue behind QK g1 on the PE.
  - projection PSUM tiles borrow scores-pool slots (NOT aux): aux holds the
    live pv accumulator, and a second rotating aux tenant serializes every
    projection against its DVE bias-add.
  - rejected on same-process HW A/B: DVE exp offload (BASS_DVE_NGROUPS=2,
    deg-2+double-squaring poly on the last 2 groups/iter) measured 579 vs
    553us — the DVE chain latency lands on the iteration transition.
    BASS_PAIR=1 (QK emission pairing across group boundaries) also negative.
  - PSUM budget: scores 2 bufs x 3 banks + pv 1 + tr 1 = 8 (full).
  - measurement: cross-process HW timing drifts ~+-8%; only same-process
    interleaved A/B slopes (ab.py / ab2.py) are trustworthy.
"""

import os
import sys

if "/opt/trn_rl_repo" not in sys.path:
    sys.path.insert(0, "/opt/trn_rl_repo")

from contextlib import ExitStack

import ml_dtypes
import numpy as np

import concourse.bass as bass
import concourse.tile as tile
from concourse import bacc, mybir
from concourse.bass_utils import run_bass_kernel_spmd
from concourse.masks import make_identity

F32 = mybir.dt.float32
# fp16 instead of bf16: all on-chip value ranges here are tiny (|x|<6,
# |W|<0.12, probs<8), so fp16's 10 mantissa bits cut quantization error ~4x
# at identical PE throughput (1 cycle/row) and xbar 2-byte transpose support
BF16 = mybir.dt.float16
AF = mybir.ActivationFunctionType
ALU = mybir.AluOpType
BF16_NP = np.float16

B, S, D, H, DK = 2, 4096, 768, 12, 64
N_CORES = 8
HPG = 3            # heads per core
GD = HPG * DK      # 192 output columns per core
SQ = 512           # q-chunk width
NSQ = S // SQ      # 8
KCW = 128          # k-chunk width
NKC = S // KCW     # 32
GRP = 3            # k-chunks per exp group (3 PSUM banks, double buffered)
NDC = D // 128     # 6 contraction chunks
QTR = S // 4       # transpose/projection pipeline granularity
SQQ = NSQ // 4     # q chunks per quarter
SCQ = NKC // 4     # s chunks per quarter


def _emit(ctx: ExitStack, tc: tile.TileContext, io: dict):
    nc = tc.nc

    const = ctx.enter_context(tc.tile_pool(name="const", bufs=1))
    # 8 slots: quarters xk0-3/xq0/xv0-2 are all live early; xv3 (9th alloc)
    # then reuses xk0's slot, whose readers finish by ~f6 — reuse of any
    # later slot (e.g. xq0's, read until ~f30) would deadlock the PE queue
    # against iteration 0's PV(g8).
    xt_pool = ctx.enter_context(tc.tile_pool(name="xt", bufs=8))
    proj = ctx.enter_context(tc.tile_pool(name="proj", bufs=1))
    scores_pool = ctx.enter_context(tc.tile_pool(name="scores", bufs=2, space="PSUM"))
    aux_psum = ctx.enter_context(tc.tile_pool(name="auxp", bufs=2, space="PSUM"))
    probs_pool = ctx.enter_context(tc.tile_pool(name="probs", bufs=5))
    outt_pool = ctx.enter_context(tc.tile_pool(name="outt", bufs=2))
    small = ctx.enter_context(tc.tile_pool(name="small", bufs=2))
    oslab_pool = ctx.enter_context(tc.tile_pool(name="oslab", bufs=3))
    # DVE-exp offload scratch: x tile, Horner ping-pong, and probs output
    dx_pool = ctx.enter_context(tc.tile_pool(name="dx", bufs=2))
    dh_pool = ctx.enter_context(tc.tile_pool(name="dh", bufs=3))
    prd_pool = ctx.enter_context(tc.tile_pool(name="prd", bufs=2))

    # ---- constants / small inputs (consolidated to limit 4KB slot padding) ----
    # mask -> per-k scale em = exp(-1e4 * (1 - mask)), [128, 32] (p, kchunk).
    # Emitted FIRST so the ACT exp-table load lands at the head of the queues.
    # All const DMAs go on the SAME queue (SP) that later issues the x
    # DMA-transposes: the shared HWDGE serializes globally and every
    # copy<->transpose transition costs an xbar-mode drain, so mixed-queue
    # issue (copies from ACT, transposes from SP) interleaved them worst-case.
    mask_em = const.tile([128, 65], F32, name="mask_em")
    mask_t = mask_em[:, 0:32]
    em_sb = mask_em[:, 32:64]
    neg1e4 = mask_em[:, 64:65]
    nc.gpsimd.memset(neg1e4, -10000.0)
    nc.sync.dma_start(mask_t, io["mask_pk"][:])
    nc.scalar.activation(em_sb, mask_t, AF.Exp, scale=10000.0, bias=neg1e4)

    # "0": all-fp16 projections. "1": q AND k projections via fp8 DoubleRow
    # (hw-measured end-to-end rel err 1.8e-2 — too close to the 2e-2 gate).
    # "k": only the K projection in fp8 (err ~1.2e-2 l2 / 1.5e-2 absmax,
    # comfortable margin) at half the PE savings.
    FP8MODE = __import__("os").environ.get("BASS_FP8QK", "k")
    FP8QK = FP8MODE in ("1", "k")
    F8 = mybir.dt.float8e4

    def fp8_for(wi):
        return FP8MODE == "1" or (FP8MODE == "k" and wi == 1)

    if FP8QK:
        # q/k weights as fp8 DoubleRow pairs, host-packed in tile layout:
        # (ki, c, i, j, h, m) = 32*W_i[256c + 2ki + j, h*64 + m%64]
        # (x32 lifts W sigma=0.02 out of fp8's subnormal range; the bias-add
        # multiplies the PSUM result by 1/32)
        w8 = const.tile([128, 3, 2, 2, HPG, 128], F8, name="w8")
        nc.sync.dma_start(
            w8[:],
            io["wqk8"].rearrange(
                "p (c i j h m) -> p c i j h m", c=3, i=2, j=2, h=HPG
            ),
        )
    # fp16 weight slabs: v always; q and/or k when their projection is fp16
    fp16_w = [(2, "wv")]
    if not fp8_for(0):
        fp16_w.append((0, "wq"))
    if not fp8_for(1):
        fp16_w.append((1, "wk"))
    w_all = const.tile([128, NDC, 3 * GD], BF16, name="w_all")
    for i, nm in fp16_w:
        nc.sync.dma_start(
            w_all[:, :, i * GD : (i + 1) * GD],
            io[nm].rearrange("(dc p) n -> p dc n", p=128),
        )
    wv_sb = w_all[:, :, 2 * GD : 3 * GD]

    if not (fp8_for(0) and fp8_for(1)):
        # q/k weights with each head's 64 columns duplicated (projection then
        # replicates qT/kT on both partition halves at no extra PE cost)
        w_dup = const.tile([128, NDC, 2, HPG, 128], BF16, name="w_dup")
        for i, _nm in fp16_w:
            if i == 2:
                continue
            for h in range(HPG):
                for rep in range(2):
                    nc.vector.tensor_copy(
                        w_dup[:, :, i, h, rep * DK : (rep + 1) * DK],
                        w_all[:, :, i * GD + h * DK : i * GD + (h + 1) * DK],
                    )

    bqbk = const.tile([128, 2 * HPG], F32, name="bqbk")
    nc.sync.dma_start(bqbk[:], io["bqbk_pk"][:])

    bfpack = const.tile([1, 320], BF16, name="bfpack")
    nc.gpsimd.memset(bfpack[:, 0:128], 1.0)
    nc.sync.dma_start(bfpack[:, 128 : 128 + GD], io["bv_r"][:])
    ones_row = bfpack[:, 0:128]
    bv_sb = bfpack[:, 128 : 128 + GD]

    ident = const.tile([128, 128], F32, name="ident")
    make_identity(nc, ident[:])

    # ---- persistent projection outputs (qT/kT replicated on both halves) ----
    qT = proj.tile([128, HPG, S], BF16, name="qT")
    kT = proj.tile([128, HPG, S], BF16, name="kT")
    vE = proj.tile([128, NKC, HPG, DK + 1], BF16, name="vE")
    nc.gpsimd.memset(vE[:], 1.0)  # ones col 64; data cols overwritten below

    # ---- per-quarter: transpose + project ----
    def load_xt_quarter(nm, qq):
        # host supplies x d-chunk-major [6*4096, 128] so each xbar transpose
        # reads a fully contiguous [1024, 128] block. In FP8QK mode, xq/xk
        # arrive byte-packed (two fp8 d-neighbors per uint16 element): 3
        # chunks of 128 pair-columns, half the DMA bytes.
        packed = (nm == "xk" and FP8QK) or (nm == "xq" and FP8MODE == "1")
        nch = 3 if packed else NDC
        xt = xt_pool.tile([128, nch, QTR], BF16, tag="xt", name=f"xt_{nm}_{qq}")
        for dc in range(nch):
            base = dc * S + qq * QTR
            nc.sync.dma_start(
                out=xt[:, dc, :], in_=io[nm][base : base + QTR, :],
                transpose=True,
            )
        return xt

    def proj_qk_one(xt, qq, wi, bias, dst, h, sqq):
        # scores-pool slot (not aux): aux holds the live pv accumulator, so a
        # second rotating tenant there would serialize every projection
        # against its DVE bias-add read
        sq = qq * SQQ + sqq
        ps = scores_pool.tile([128, SQ], F32, tag="scores", name=f"ps_{wi}_{qq}_{h}_{sqq}")
        if fp8_for(wi):
            # fp8 DoubleRow: 3 contraction chunks of 256 d (pairs d=256c+2ki+j
            # matching the byte-packed transpose and the host w8 layout);
            # each chunk streams N=512 at 0.5 cycles/row
            x8 = xt[:].bitcast(F8).rearrange("p c (s j) -> p c j s", j=2)
            for c in range(3):
                nc.tensor.matmul(
                    ps[:],
                    lhsT=w8[:, c, wi, :, h, :],
                    rhs=x8[:, c, :, sqq * SQ : (sqq + 1) * SQ],
                    start=(c == 0),
                    stop=(c == 2),
                    perf_mode=mybir.MatmulPerfMode.DoubleRow,
                )
            # undo the x32 weight scale, then add bias
            nc.vector.tensor_scalar(
                dst[:, h, sq * SQ : (sq + 1) * SQ], ps[:],
                0.03125, bias[:, h : h + 1], ALU.mult, ALU.add,
            )
        else:
            for dc in range(NDC):
                nc.tensor.matmul(
                    ps[:],
                    lhsT=w_dup[:, dc, wi, h, :],
                    rhs=xt[:, dc, sqq * SQ : (sqq + 1) * SQ],
                    start=(dc == 0),
                    stop=(dc == NDC - 1),
                )
            nc.vector.tensor_scalar(
                dst[:, h, sq * SQ : (sq + 1) * SQ], ps[:],
                bias[:, h : h + 1], None, ALU.add,
            )

    def proj_qk(xt, qq, wi, bias, dst, skip=None):
        for h in range(HPG):
            for sqq in range(SQQ):
                if skip is not None and (h, sqq) in skip:
                    continue
                proj_qk_one(xt, qq, wi, bias, dst, h, sqq)

    def proj_v(xt, qq, scqs=None):
        for scq in (range(SCQ) if scqs is None else scqs):
            sc = qq * SCQ + scq
            ps = scores_pool.tile([128, GD], F32, tag="scores", name=f"psv_{qq}_{scq}")
            for dc in range(NDC):
                nc.tensor.matmul(
                    ps[:],
                    lhsT=xt[:, dc, scq * 128 : (scq + 1) * 128],
                    rhs=wv_sb[:, dc, :],
                    start=(dc == 0),
                    stop=False,
                )
            nc.tensor.matmul(
                ps[:], lhsT=ones_row[:, 0:128], rhs=bv_sb[:], start=False, stop=True
            )
            for h in range(HPG):
                nc.vector.tensor_copy(
                    vE[:, sc, h, 0:DK], ps[:, h * DK : (h + 1) * DK]
                )
            # fold mask scale into v and the denominator ones column
            nc.vector.tensor_scalar(
                vE[:, sc, :, :], vE[:, sc, :, :], em_sb[:, sc : sc + 1], None,
                ALU.mult,
            )

    def proj_q_group(xt, qq, h, sqq):
        proj_qk_one(xt, qq, 0, bqbk[:, 0:HPG], qT, h, sqq)

    # ---- attention ----
    groups = []
    g0 = 0
    while g0 < NKC:
        groups.append((g0, min(GRP, NKC - g0)))
        g0 += GRP

    # ScalarE exp is the kernel's critical engine in steady state. Offload
    # the LAST `DVE_NGROUPS` exp groups of each iteration (latest PV
    # deadlines) to a DVE polynomial: exp(s/8) = h(y)^4 with y = s/32 and
    # h a degree-2 least-squares fit of exp on y in [-0.5, 0.5]. Squaring
    # twice keeps probs nonnegative by construction. 6 DVE ops per group
    # (1 PSUM-read + 5 fp16-SBUF ops) vs 1 ACT op; worth it because DVE is
    # ~80% idle while ACT is the wall. Adds ~3.7e-3 relative error on the
    # offloaded 5/32 of keys (budget 2e-2).
    # A/B on HW (same-process, interleaved): offload=2 groups measured ~26us
    # SLOWER (579 vs 553) despite ACT being the busiest engine — the DVE
    # chain latency sits on the iteration-transition critical path. Off by
    # default.
    DVE_NGROUPS = int(__import__("os").environ.get("BASS_DVE_NGROUPS", "0"))
    C0, C1, C2 = 1.00148143, 1.02379966, 0.48757841

    def emit_dve_exp(sc, pr, w, it, gi):
        yt = dx_pool.tile([128, w], BF16, tag="dx", name=f"y_{it}_{gi}")
        nc.vector.tensor_scalar(yt[:], sc[:], 0.03125, None, ALU.mult)
        t1 = dh_pool.tile([128, w], BF16, tag="dh", name=f"t1_{it}_{gi}")
        nc.vector.tensor_scalar(t1[:], yt[:], C2, C1, ALU.mult, ALU.add)
        t2 = dh_pool.tile([128, w], BF16, tag="dh", name=f"t2_{it}_{gi}")
        nc.vector.tensor_tensor(t2[:], t1[:], yt[:], ALU.mult)
        t3 = dh_pool.tile([128, w], BF16, tag="dh", name=f"t3_{it}_{gi}")
        nc.vector.tensor_scalar(t3[:], t2[:], C0, None, ALU.add)
        t4 = dh_pool.tile([128, w], BF16, tag="dh", name=f"t4_{it}_{gi}")
        nc.vector.tensor_tensor(t4[:], t3[:], t3[:], ALU.mult)
        nc.vector.tensor_tensor(pr[:], t4[:], t4[:], ALU.mult)

    pending = None  # finalize closure for the previous (h, sq)

    # finalize split in two: part 1 (the pv->SBUF copy, which frees the pv
    # PSUM slot) fires at gi==0 so it lands in the DVE queue BEFORE the
    # ~10us exp-poly chain; part 2 (PE transposes + normalize + store) at
    # gi==1 so the transposes sit behind QK g1 in the PE queue and never
    # stall on the copy.
    def make_fin_copy(pv, h, sq):
        ot = outt_pool.tile([DK + 1, SQ], F32, tag="outt", name=f"ot_{h}_{sq}")

        def fin1():
            nc.vector.tensor_copy(ot[:], pv[:])
        return ot, fin1

    def make_finalize(ot, h, sq):
        def fin():
            tr = aux_psum.tile([128, 4 * (DK + 1)], F32, tag="aux", name=f"tr_{h}_{sq}")
            for t in range(4):
                nc.tensor.transpose(
                    tr[:, t * (DK + 1) : (t + 1) * (DK + 1)],
                    ot[:, t * 128 : (t + 1) * 128],
                    ident[0 : DK + 1, 0 : DK + 1],
                )
            rc = small.tile([128, 4], F32, tag="recip", name=f"rc_{h}_{sq}")
            osl = oslab_pool.tile([128, 4, DK], F32, tag="oslab", name=f"os_{h}_{sq}")
            for t in range(4):
                nc.vector.reciprocal(
                    rc[:, t : t + 1], tr[:, t * (DK + 1) + DK : t * (DK + 1) + DK + 1]
                )
                nc.vector.tensor_scalar(
                    osl[:, t, :],
                    tr[:, t * (DK + 1) : t * (DK + 1) + DK],
                    rc[:, t : t + 1],
                    None,
                    ALU.mult,
                )
            nc.gpsimd.dma_start(
                out=io["out"].rearrange(
                    "(sq t p) n -> sq p t n", sq=NSQ, t=4, p=128
                )[sq, :, :, h * DK : (h + 1) * DK],
                in_=osl[:],
            )
        return fin

    # Boundary tasks: kept as an (empty by default) hook used by the carry
    # emission point inside attention_gen.
    boundary_tasks = {}
    xt_q_tiles = {}

    def attention_gen():
        nonlocal_pending = [None]

        def emit_pv(pv, h, grp, pvst):
            # start/stop are positional (pvst counts PV matmuls emitted for
            # this accumulator): chunk emission order is permuted when DVE
            # exp groups are deferred, so kc == 0 is not necessarily first.
            p0, plen, ppr = grp
            for j in range(plen):
                kc = p0 + j
                nc.tensor.matmul(
                    pv[:],
                    lhsT=vE[:, kc, h, :],
                    rhs=ppr[:, j * SQ : (j + 1) * SQ],
                    start=(pvst[0] == 0),
                    stop=(pvst[0] == NKC - 1),
                )
                pvst[0] += 1

        carry = None  # (pv, h, [groups]) tail-PV work carried across iterations
        it = 0
        # PV pop threshold (lag in exp groups before a group's PV is emitted).
        # 3 measured -28us on HW vs 2 (sim agrees: the PV tail decouples from
        # the iteration transition; carry grows to 3 groups, emitted at the
        # next iteration's start where ACT is busy anyway).
        PVLAG = int(__import__("os").environ.get("BASS_PVLAG", "3"))
        ITER_HMINOR = __import__("os").environ.get("BASS_HMINOR", "1") == "1"
        # h-minor / sq-major order: qT quarter q is first needed at iteration
        # 3*2q, so the xq DMAs and qT projections can trail far behind the
        # kT/vE pipeline instead of gating it.
        if ITER_HMINOR:
            iters = [(h, sq) for sq in range(NSQ) for h in range(HPG)]
        else:
            iters = [(h, sq) for h in range(HPG) for sq in range(NSQ)]
        for h, sq in iters:
            # From iteration 2 on, the DVE-offloaded groups (the LAST k-chunk
            # groups) are FIRED FIRST: their scores slots free early (from the
            # previous iteration), and their ~5us DVE poly chains overlap this
            # iteration's ACT burst instead of delaying the carry PVs.
            offload = DVE_NGROUPS if it >= 2 else 0
            if offload:
                act_groups = groups[-offload:] + groups[:-offload]
            else:
                act_groups = groups
            dve_set = set(range(offload))
            pv = aux_psum.tile([DK + 1, SQ], F32, tag="aux", name=f"pv_{h}_{sq}")
            pvst = [0]
            ready = []  # (kc0, glen, probs) groups awaiting PV emission
            defer = []  # (eligible_fire_idx, grp) DVE groups awaiting poly
            # QK matmuls are emitted in strict (even, odd) kc pairs ACROSS
            # group boundaries so every matmul lands adjacent to its
            # opposite-row-group partner in the PE queue and the two K=64
            # halves run concurrently (PV blocks between groups would
            # otherwise orphan each group's 3rd chunk)
            chunk_list = []
            for gi, (kc0, glen) in enumerate(act_groups):
                for j in range(glen):
                    chunk_list.append((kc0 + j, gi, j))
            sc_tiles = {}
            filled = [0] * len(act_groups)
            fired = 0
            ci = 0
            # no pairing in iteration 0: its one-chunk lookahead would hold a
            # live scores tile across the prologue's advance() points, where
            # proj_q_group borrows slots from the same pool
            # A/B on HW: pairing measured neutral-to-worse (478us vs 450us
            # best-valid samples) — likely the 1-chunk lookahead couples the
            # PE to the previous exp via the scores double-buffer. Opt-in.
            do_pair = __import__("os").environ.get("BASS_PAIR", "0") == "1"
            pair_n = 2 if (it > 0 and do_pair) else 1
            while ci < len(chunk_list):
                for _ in range(pair_n):
                    if ci >= len(chunk_list):
                        break
                    kc, gi, j = chunk_list[ci]
                    ci += 1
                    if gi not in sc_tiles:
                        sc_tiles[gi] = scores_pool.tile(
                            [128, act_groups[gi][1] * SQ], F32, tag="scores",
                            name=f"sc_{h}_{sq}_{gi}",
                        )
                    ho = 64 * (kc % 2)
                    nc.tensor.matmul(
                        sc_tiles[gi][:, j * SQ : (j + 1) * SQ],
                        lhsT=kT[ho : ho + DK, h, kc * KCW : (kc + 1) * KCW],
                        rhs=qT[ho : ho + DK, h, sq * SQ : (sq + 1) * SQ],
                        start=True,
                        stop=True,
                    )
                    filled[gi] += 1
                while fired < len(act_groups) and (
                    filled[fired] == act_groups[fired][1]
                ):
                    gi = fired
                    kc0, glen = act_groups[gi]
                    pr = probs_pool.tile(
                        [128, glen * SQ], BF16, tag="probs",
                        name=f"pr_{h}_{sq}_{gi}",
                    )
                    if gi in dve_set:
                        emit_dve_exp(
                            sc_tiles.pop(gi), pr, glen * SQ, it, gi
                        )
                        defer.append((gi + 4, (kc0, glen, pr)))
                    else:
                        nc.scalar.activation(
                            pr[:], sc_tiles.pop(gi)[:], AF.Exp, scale=0.125
                        )
                        ready.append((kc0, glen, pr))
                    while defer and defer[0][0] <= gi:
                        ready.append(defer.pop(0)[1])
                    if gi == 0:
                        if carry is not None:
                            cpv, ch, cgrps, cpvst = carry
                            for grp in cgrps:
                                emit_pv(cpv, ch, grp, cpvst)
                            carry = None
                            for fn in boundary_tasks.get(it - 1, ()):
                                fn()
                        # free the previous pv PSUM slot (DVE copy) BEFORE
                        # the iteration's poly chains enter the DVE queue
                        if nonlocal_pending[0] is not None:
                            nonlocal_pending[0][0]()
                    if gi == 1 and nonlocal_pending[0] is not None:
                        nonlocal_pending[0][1]()
                        nonlocal_pending[0] = None
                    # iteration 0 holds one extra group in flight: its vE
                    # quarters are still streaming in, so the deeper lag keeps
                    # PV emission behind the xv DMA wavefront
                    if len(ready) >= (max(3, PVLAG) if it == 0 else PVLAG):
                        emit_pv(pv, h, ready.pop(0), pvst)
                    fired += 1
                    yield (h, sq, gi)
            carry_grps = list(ready) + [g for _, g in defer]
            carry = (pv, h, carry_grps, pvst)
            ot, fin1 = make_fin_copy(pv, h, sq)
            nonlocal_pending[0] = (fin1, make_finalize(ot, h, sq))
            it += 1

        cpv, ch, cgrps, cpvst = carry
        for grp in cgrps:
            emit_pv(cpv, ch, grp, cpvst)
        nonlocal_pending[0][0]()
        nonlocal_pending[0][1]()

    # All x DMA-transposes are issued upfront in availability order so the
    # DMA engines stream continuously from t=0 (the quarter-chasing scheme
    # left them ~50% idle over a 100us window). kT quarters land first (they
    # gate iteration 0's exp groups), vE quarters interleave (they gate the
    # trailing PV), and xq quarters come last (h-minor iteration order defers
    # the first sq>=2 iteration far enough that qT projection can trail).
    gen = attention_gen()

    def advance(n):
        for _ in range(n):
            if next(gen, None) is None:
                break

    bk = bqbk[:, HPG : 2 * HPG]
    xt_k0 = load_xt_quarter("xk", 0)
    xt_q_tiles[0] = load_xt_quarter("xq", 0)
    xt_v0 = load_xt_quarter("xv", 0)
    xt_k1 = load_xt_quarter("xk", 1)
    xt_v1 = load_xt_quarter("xv", 1)
    xt_k2 = load_xt_quarter("xk", 2)
    xt_v2 = load_xt_quarter("xv", 2)
    xt_k3 = load_xt_quarter("xk", 3)
    # (xv3 + xq1..3 are issued below once xt slots have been consumed)

    # Fine-grained prologue: projections are emitted in DMA-availability
    # order, spread across iteration 0's fires so the PE never holds a long
    # burst ahead of runnable attention work, and per-(head, sqq) so only
    # head 0's kT gates iteration 0 (h1/h2 trail for iterations 1-2; all xt
    # readers still finish within iteration 0's span, freeing slots for
    # xv3/xq1-3).
    # PV(g) of iteration 0 is emitted at fire g+3 (pop threshold 3), so vE
    # chunks 3g..3g+2 must be emitted before fire g+3.
    proj_qk_one(xt_k0, 0, 1, bk, kT, 0, 0)
    proj_q_group(xt_q_tiles[0], 0, 0, 0)
    advance(1)   # f1: g0 (chunks 0-2)
    proj_qk_one(xt_k0, 0, 1, bk, kT, 0, 1)
    proj_qk_one(xt_k0, 0, 1, bk, kT, 1, 0)
    advance(1)   # f2: g1
    proj_v(xt_v0, 0, (0, 1, 2))
    proj_qk_one(xt_k1, 1, 1, bk, kT, 0, 0)    # g2's chunk 8
    advance(1)   # f3: g2 + PV(g0)
    proj_v(xt_v0, 0, (3, 4, 5))
    proj_qk_one(xt_k0, 0, 1, bk, kT, 1, 1)
    advance(1)   # f4: g3 + PV(g1)
    proj_v(xt_v0, 0, (6, 7))
    proj_v(xt_v1, 1, (0,))
    proj_qk_one(xt_k1, 1, 1, bk, kT, 0, 1)    # g4's chunks 12-14
    proj_qk_one(xt_k0, 0, 1, bk, kT, 2, 0)
    advance(1)   # f5: g4 + PV(g2)
    proj_v(xt_v1, 1, (1, 2, 3))
    proj_qk_one(xt_k2, 2, 1, bk, kT, 0, 0)    # g5's chunks 16-17
    proj_qk_one(xt_k0, 0, 1, bk, kT, 2, 1)
    advance(1)   # f6: g5 + PV(g3)
    proj_v(xt_v1, 1, (4, 5, 6))
    proj_qk_one(xt_k2, 2, 1, bk, kT, 0, 1)    # g6's chunk 20
    proj_qk_one(xt_k1, 1, 1, bk, kT, 1, 0)
    advance(1)   # f7: g6 + PV(g4)
    xt_v3 = load_xt_quarter("xv", 3)
    proj_v(xt_v1, 1, (7,))
    proj_v(xt_v2, 2, (0, 1))
    proj_qk_one(xt_k1, 1, 1, bk, kT, 1, 1)
    advance(1)   # f8: g7 + PV(g5)
    proj_v(xt_v2, 2, (2, 3, 4))
    proj_qk_one(xt_k3, 3, 1, bk, kT, 0, 0)    # g8's chunk 24
    proj_qk_one(xt_k1, 1, 1, bk, kT, 2, 0)
    advance(1)   # f9: g8 + PV(g6)
    proj_v(xt_v2, 2, (5, 6, 7))
    proj_qk_one(xt_k3, 3, 1, bk, kT, 0, 1)    # g9's chunk 28
    proj_qk_one(xt_k1, 1, 1, bk, kT, 2, 1)
    xt_q_tiles[1] = load_xt_quarter("xq", 1)
    advance(1)   # f10: g9 + PV(g7)
    proj_v(xt_v3, 3, (0, 1, 2))
    proj_qk_one(xt_k2, 2, 1, bk, kT, 1, 0)
    proj_qk_one(xt_k2, 2, 1, bk, kT, 1, 1)
    xt_q_tiles[2] = load_xt_quarter("xq", 2)
    advance(1)   # f11: g10 + PV(g8) — iteration 0 groups complete
    proj_v(xt_v3, 3, (3, 4, 5, 6, 7))
    proj_q_group(xt_q_tiles[0], 0, 1, 0)      # iter1 = (h1, s0) at f12
    proj_qk_one(xt_k2, 2, 1, bk, kT, 2, 0)
    proj_qk_one(xt_k2, 2, 1, bk, kT, 2, 1)
    xt_q_tiles[3] = load_xt_quarter("xq", 3)
    advance(2)   # f12 (iter1 g0: carry PVs g9,g10 + fin), f13
    proj_qk_one(xt_k3, 3, 1, bk, kT, 1, 0)    # iter1 g8 reads at f20
    proj_qk_one(xt_k3, 3, 1, bk, kT, 1, 1)
    advance(2)   # f14, f15
    proj_qk_one(xt_k3, 3, 1, bk, kT, 2, 0)    # iter2 g8 reads at f31
    proj_qk_one(xt_k3, 3, 1, bk, kT, 2, 1)
    proj_q_group(xt_q_tiles[0], 0, 2, 0)      # iter2 = (h2, s0) at f23
    advance(7)   # f16-f22
    proj_q_group(xt_q_tiles[0], 0, 0, 1)      # iter3 = (h0, s1) at f34
    advance(2)   # f23, f24
    proj_q_group(xt_q_tiles[0], 0, 1, 1)      # iter4 at f45
    advance(5)   # f25-f29
    proj_q_group(xt_q_tiles[0], 0, 2, 1)      # iter5 at f56
    advance(5)   # f30-f34
    advance(10)  # f35-f44: let iterations 2-3 run un-surcharged (the
    # prologue's PE overflow concentrates here otherwise)
    # qT quarters 1-3: one projection per 5 fires from ~f45 (deadline for
    # quarter q head h is fire 11*(6q+h): (1,0,0) at f66, still ahead)
    for qq in (1, 2, 3):
        for h in range(HPG):
            for sqq in range(SQQ):
                proj_q_group(xt_q_tiles[qq], qq, h, sqq)
                advance(5)
    for _ in gen:
        pass


def _build():
    nc = bacc.Bacc("TRN2", target_bir_lowering=False, debug=False)
    mode = os.environ.get("BASS_FP8QK", "k")
    io = {}
    tensors = [
        ("xv", [NDC * S, 128], BF16), ("wv", [D, GD], BF16),
        ("bqbk_pk", [128, 2 * HPG], F32),
        ("bv_r", [1, GD], BF16), ("mask_pk", [128, NKC], F32),
    ]
    if mode in ("1", "k"):
        tensors.append(("wqk8", [128, 3 * 2 * 2 * HPG * 128], mybir.dt.float8e4))
        tensors.append(("xk", [3 * S, 128], BF16))
    else:
        tensors += [("xk", [NDC * S, 128], BF16), ("wk", [D, GD], BF16)]
    if mode == "1":
        tensors.append(("xq", [3 * S, 128], BF16))
    else:
        tensors += [("xq", [NDC * S, 128], BF16), ("wq", [D, GD], BF16)]
    for nm, shape, dt in tensors:
        io[nm] = nc.dram_tensor(nm, shape, dt, kind="ExternalInput").ap()
    io["out"] = nc.dram_tensor("out", [S, GD], F32, kind="ExternalOutput").ap()

    dup = int(os.environ.get("BASS_DUP", "1"))
    with tile.TileContext(nc) as tc:
        for _ in range(dup):
            with ExitStack() as ctx:
                _emit(ctx, tc, io)
    nc.compile()
    return nc


_NC = None


def _get_nc():
    global _NC
    if _NC is None:
        _NC = _build()
    return _NC


F8_NP = None


def _f8np():
    global F8_NP
    if F8_NP is None:
        from concourse import mybir as _mb
        F8_NP = _mb.dt.np(_mb.dt.float8e4)
    return F8_NP


def _pack_x8(a):
    # [S, D] f32 -> fp8 -> byte-pair uint16 carrier, d-pair-chunk-major
    # [3*S, 128] viewed as fp16 for the 2-byte xbar transpose
    a8 = np.ascontiguousarray(np.asarray(a, np.float32)).astype(_f8np())
    u = a8.view(np.uint8).reshape(S, 384, 2).view(np.uint16).reshape(S, 3, 128)
    return np.ascontiguousarray(
        u.transpose(1, 0, 2).reshape(3 * S, 128)
    ).view(np.float16)


def _pack_wqk8(Wq_c, Wk_c):
    # (ki, c, i, j, h, m) = 32 * W_i[256c + 2ki + j, h*64 + m%64]
    out = np.empty((128, 3, 2, 2, HPG, 128), np.float32)
    for i, W in ((0, Wq_c), (1, Wk_c)):
        Wr = (np.asarray(W, np.float32) * 32.0).reshape(3, 128, 2, HPG, DK)
        Wm = np.concatenate([Wr, Wr], axis=-1)        # [c, ki, j, h, 128]
        out[:, :, i] = Wm.transpose(1, 0, 2, 3, 4)    # [ki, c, j, h, 128]
    return np.ascontiguousarray(out.reshape(128, -1)).astype(_f8np())


def make_in_maps(query, key, value, mask, Wq, bq, Wk, bk, Wv, bv):
    mode = os.environ.get("BASS_FP8QK", "k")
    bf = lambda a: np.ascontiguousarray(a).astype(BF16_NP)
    bf3 = lambda a: np.ascontiguousarray(
        np.asarray(a).reshape(S, NDC, 128).transpose(1, 0, 2).reshape(NDC * S, 128)
    ).astype(BF16_NP)
    f32 = lambda a: np.ascontiguousarray(np.asarray(a, np.float32))
    in_maps = []
    for c in range(N_CORES):
        b, g = divmod(c, 4)
        cols = slice(g * GD, (g + 1) * GD)
        m = {
            "xv": bf3(value[b]),
            "wv": bf(Wv[:, cols]),
            "bqbk_pk": f32(np.tile(np.concatenate(
                [np.asarray(bq)[cols].reshape(HPG, DK).T,
                 np.asarray(bk)[cols].reshape(HPG, DK).T], axis=1), (2, 1))),
            "bv_r": bf(np.asarray(bv)[cols].reshape(1, GD)),
            "mask_pk": f32(np.asarray(mask)[b].reshape(NKC, 128).T),
        }
        if mode in ("1", "k"):
            m["xk"] = _pack_x8(key[b])
            m["wqk8"] = _pack_wqk8(
                np.asarray(Wq)[:, cols], np.asarray(Wk)[:, cols]
            )
        else:
            m["xk"] = bf3(key[b])
            m["wk"] = bf(Wk[:, cols])
        if mode == "1":
            m["xq"] = _pack_x8(query[b])
        else:
            m["xq"] = bf3(query[b])
            m["wq"] = bf(Wq[:, cols])
        in_maps.append(m)
    return in_maps


def kernel(query, key, value, mask, Wq, bq, Wk, bk, Wv, bv):
    query = np.asarray(query, np.float32)
    key = np.asarray(key, np.float32)
    value = np.asarray(value, np.float32)
    nc = _get_nc()
    in_maps = make_in_maps(query, key, value, mask, Wq, bq, Wk, bk, Wv, bv)
    res = run_bass_kernel_spmd(nc, in_maps, core_ids=list(range(N_CORES)))
    out = np.empty((B, S, D), np.float32)
    for c in range(N_CORES):
        b, g = divmod(c, 4)
        out[b, :, g * GD : (g + 1) * GD] = res.results[c]["out"]
    return out

